# revision 1
# baseline (speedup 1.0000x reference)
"""Trainium2 Bass kernel for nn_Attn_52432960749709.

Computes, for E:[B,N,D], W1/W2:[D,D]:
    q = E @ W1 ; k = E @ W2
    scores = (q @ k^T) / sqrt(D)          # per batch, [N, N]
    out = softmax(scores, axis=1)         # normalize over rows n, per column m

Strategy (data parallel over B across 8 NeuronCores, one batch element per
core; the small DxD weights are folded on the host into M = W1 @ W2^T and
replicated):

    scores = E M E^T / sqrt(D)
    Per core (one NeuronCore per batch element):
      head    14 PE warmup transposes span the ~3us clock-ramp window while
              the first loads land (E pair 0 as two single tiles, M as four
              per-chunk DMA+f32r-round pairs, each matched to its first
              consumer's order)
      E^T     PE transposes (fp32), pipelined per E-pair with the
              G^T = M E^T f32r matmuls one pair behind
      s^T     [128 m, 512 n] f32r matmuls; ACT exp(scale*s) -> fp16 strip
              with accum_out producing Z per partition; DVE: 1/Z, then one
              4x-packed tensor_scalar normalize per chunk
      out     PE transpose-back (fp16) interleaved at skew-1 between matmul
              bursts; two adjacent m-chunks share an output strip so HBM
              writes are 512B-contiguous, shipped as two j-half DMAs
      tail    the last chunk runs quarter-granular: normalize quarter
              (DVE/ACT alternating) -> transpose quarter -> quarter DMA
    Host upcasts the fp16 output to fp32.
"""

import math

import numpy as np

B, N, D = 8, 2048, 512
P = 128
DC = D // P  # 4 contraction chunks
NB = 512  # matmul moving free dim
NBS = N // NB  # 4 n-blocks per row strip
MC = N // P  # 16 m-chunks per core

_CACHE: dict = {}

# debug: limit build to first K phases (0=all): 1=loads+ET, 2=+GT
_BUILD_PHASES = [0]


def _patch_tile_drain():
    """This walrus build rejects >1 extra sem wait on one TPB_CTRL
    instruction, so split the end-of-kernel drain's wait set across chained
    SP NOPs (same engine, so program order preserves barrier semantics)."""
    import concourse.tile as tile
    from concourse.vector_clock import ScopedClock

    if getattr(tile.TileContext, "_drain_split_patched", False):
        return

    max_waits = 1

    def _drain_and_barrier_split(self, tick_clock, wait_clock):
        nc = self.nc
        drain_inst = nc.sync.drain()
        wait_clock.add_sem_waits(
            drain_inst.ins, ScopedClock({None: tick_clock.global_clock})
        )
        si = drain_inst.ins.sync_info
        waits = list(si.on_wait or []) if si is not None else []
        if len(waits) > max_waits:
            si.on_wait = waits[:max_waits]
            rest = waits[max_waits:]
            while rest:
                nop = nc.sync.nop(nofuse=True, hint="drain_wait_split")
                chunk, rest = rest[:max_waits], rest[max_waits:]
                nsi = nop.ins.sync_info
                if nsi is None:
                    import bass_rust

                    nop.ins.sync_info = bass_rust.SyncInfo(
                        on_wait=chunk, on_update=[]
                    )
                else:
                    nsi.on_wait = chunk

        nc.all_engine_barrier()
        assert self.sems is not None
        popped = nc._tile_sem_poison_stack.pop()
        assert popped is self._sem_poison
        nc.clear_and_free_semaphores(list(self.sems.allocated().values()))
        nc.all_engine_barrier()

    tile.TileContext._drain_and_barrier = _drain_and_barrier_split
    tile.TileContext._drain_split_patched = True


def _split_multi_waits(nc):
    """This walrus build supports only one sem-wait command per instruction.
    Hoist extra waits onto same-engine NOPs inserted just before the
    instruction (engines execute in order, so semantics are preserved)."""
    import bass_rust
    import concourse.mybir as mybir

    ctr = 0
    for fn in nc.m.functions:
        for blk in fn.blocks:
            insts = blk.instructions
            out = []
            changed = False
            for inst in insts:
                si = inst.sync_info
                waits = list(si.on_wait) if (si is not None and si.on_wait) else []
                if len(waits) > 1:
                    changed = True
                    for w in waits[:-1]:
                        ctr += 1
                        nop = mybir.InstNoOp(name=f"I-waitsplit-{ctr}")
                        nop.engine = inst.engine
                        nop.sync_info = bass_rust.SyncInfo(
                            on_wait=[w], on_update=[]
                        )
                        nc.register_instruction(nop)
                        out.append(nop)
                    si.on_wait = waits[-1:]
                out.append(inst)
            if changed:
                blk.instructions = out


def _build_nc(repeat=1):
    import concourse.bass as bass
    import concourse.mybir as mybir
    import concourse.tile as tile
    from concourse.masks import make_identity

    _patch_tile_drain()

    dt = mybir.dt
    f32, f32r, f16 = dt.float32, dt.float32r, dt.float16
    Exp = mybir.ActivationFunctionType.Exp
    X = mybir.AxisListType.X

    scale = 1.0 / math.sqrt(float(D))

    nc = bass.Bass()
    E_d = nc.dram_tensor("E", [N, D], f32, kind="ExternalInput")
    M_d = nc.dram_tensor("M", [D, D], f32, kind="ExternalInput")
    O_d = nc.dram_tensor("O", [N, N], f16, kind="ExternalOutput")
    # Output viewed as [p, j, m] with n = j*128 + p for the transpose-back DMA.
    O_r = O_d[:].rearrange("(j p) m -> p j m", p=P)

    with tile.TileContext(nc) as tc:
        with (
            tc.tile_pool(name="persist", bufs=1) as persist,
            tc.tile_pool(name="ein", bufs=6) as ein,
            tc.tile_pool(name="exps", bufs=4) as exps,
            tc.tile_pool(name="outs", bufs=3) as outs,
            tc.tile_pool(name="ots", bufs=3) as otsp,
            tc.tile_pool(name="small", bufs=8) as small,
            tc.tile_pool(name="psum_s", bufs=3, space="PSUM") as psum_s,
            tc.tile_pool(name="psum_t", bufs=2, space="PSUM") as psum_t,
        ):
            ident32 = persist.tile([P, P], f32, tag="id32")
            make_identity(nc, ident32)
            ident16 = persist.tile([P, P], f16, tag="id16")
            make_identity(nc, ident16)

            # PE warmup: keep the PE busy while the first E tiles stream in,
            # so the HAM clock gate is released before real work arrives.
            warm = psum_s.tile([P, 2 * NB], f32, tag="ps", name="warm")
            for k in range(14):
                nc.tensor.transpose(
                    warm[:, (k % 4) * P : (k % 4 + 1) * P], ident32, ident32
                )

            # f32r: fp32 storage pre-rounded for single-pass PE matmul; the
            # producing copy instruction performs the rounding.
            ET = persist.tile([P, DC, N], f32r, tag="ET")  # E^T  [d, n]
            GT = persist.tile([P, DC, N], f32r, tag="GT")  # G^T  [d', n]
            Mraw = persist.tile([P, DC, D], f32, tag="Mraw")
            Msb = persist.tile([P, DC, D], f32r, tag="M")  # M    [d, d']

            m_loaded = [False]

            def load_M():
                # per-chunk loads + rounds so gt_pair(0)'s dc=0 weights are
                # ready ~4us earlier than with one monolithic 1MB load
                if m_loaded[0]:
                    return
                m_loaded[0] = True
                for c in range(DC):
                    nc.sync.dma_start(Mraw[:, c, :], M_d[c * P : (c + 1) * P, :])
                    nc.vector.tensor_copy(out=Msb[:, c, :], in_=Mraw[:, c, :])

            def emit_once():
                # ---- E^T (PE transposes) with G^T = M E^T interleaved ----
                # E tiles processed in pairs -> 256 E^T columns at a time; the
                # pair's GT columns (all d') are computed one pair behind the
                # transposes so PE never waits on the DVE copies.
                W2C = 2 * P  # 256 columns per pair

                egroups = {}

                def load_group(g):
                    eg = ein.tile([P, 2, D], f32, tag="eg", name="eg")
                    nc.sync.dma_start(
                        eg,
                        E_d[g * 2 * P : (g + 1) * 2 * P, :].rearrange(
                            "(o p) d -> p o d", p=P
                        ),
                    )
                    egroups[g] = eg

                def load_single(i):
                    e1 = ein.tile([P, D], f32, tag="eg1", name="e1")
                    nc.sync.dma_start(e1, E_d[i * P : (i + 1) * P, :])
                    return e1

                def et_pair(pi):
                    if pi == 0:
                        # first pair as two half-size loads so the first
                        # transposes start ~0.7us earlier; M queues behind them
                        singles = [load_single(0), load_single(1)]
                        load_M()
                    for gg in range(max(pi, 1), min(pi + 3, MC // 2)):
                        if gg not in egroups:
                            load_group(gg)
                    ps = psum_s.tile([P, 2 * NB], f32, tag="ps", name="ps_et")
                    for half in range(2):
                        o = half
                        src = (
                            singles[half][:, :]
                            if pi == 0
                            else egroups[pi][:, o, :]
                        )
                        for j in range(DC):
                            nc.tensor.transpose(
                                ps[:, j * W2C + half * P : j * W2C + (half + 1) * P],
                                src[:, j * P : (j + 1) * P],
                                ident32,
                            )
                    nc.scalar.copy(
                        out=ET[:, :, pi * W2C : (pi + 1) * W2C],
                        in_=ps.rearrange("p (c n) -> p c n", c=DC),
                    )

                def gt_pair(pi):
                    ps = psum_s.tile([P, 2 * NB], f32, tag="ps", name="ps_gt")
                    for dpc in range(DC):
                        for dc in range(DC):
                            nc.tensor.matmul(
                                ps[:, dpc * W2C : (dpc + 1) * W2C],
                                lhsT=Msb[:, dc, dpc * P : (dpc + 1) * P],
                                rhs=ET[:, dc, pi * W2C : (pi + 1) * W2C],
                                start=(dc == 0),
                                stop=(dc == DC - 1),
                            )
                    nc.vector.tensor_copy(
                        out=GT[:, :, pi * W2C : (pi + 1) * W2C],
                        in_=ps.rearrange("p (c n) -> p c n", c=DC),
                    )

                NPAIR = MC // 2
                if _BUILD_PHASES[0] == 1:
                    for pi in range(NPAIR):
                        et_pair(pi)
                else:
                    for pi in range(NPAIR):
                        et_pair(pi)
                        if pi >= 1:
                            gt_pair(pi - 1)
                    gt_pair(NPAIR - 1)

                # ---- main loop over m-chunks, software-pipelined ----
                # stage a(mi): scores matmuls + exp pass1
                # stage b(mi): 1/Z + normalize pass2       (emitted at iter mi+1)
                # stage c(mi): transpose-back + copies + DMA (emitted at iter mi+2)
                ssbs: dict = {}
                osbs: dict = {}
                apart: dict = {}

                def stage_a_half(mi, h):
                    if mi not in apart:
                        ssb = exps.tile([P, N], f16, tag="ssb", name="ssb")
                        zp = small.tile([P, 2], f32, tag="zp", name="zp")
                        apart[mi] = (ssb, zp)
                    ssb, zp = apart[mi]
                    ps = psum_s.tile([P, 2 * NB], f32, tag="ps", name="ps_a")
                    for q in range(2):
                        nb = 2 * h + q
                        for dc in range(DC):
                            nc.tensor.matmul(
                                ps[:, q * NB : (q + 1) * NB],
                                lhsT=ET[:, dc, mi * P : (mi + 1) * P],
                                rhs=GT[:, dc, nb * NB : (nb + 1) * NB],
                                start=(dc == 0),
                                stop=(dc == DC - 1),
                            )
                    nc.scalar.activation(
                        ssb[:, h * 2 * NB : (h + 1) * 2 * NB],
                        ps,
                        Exp,
                        scale=scale,
                        accum_out=zp[:, h : h + 1],
                    )

                def stage_a_fin(mi):
                    ssb, zp = apart.pop(mi)
                    zs = small.tile([P, 1], f32, tag="zs")
                    nc.vector.reduce_sum(zs, zp, axis=X)
                    rv = small.tile([P, 1], f32, tag="rv")
                    nc.vector.reciprocal(rv, zs)
                    ssbs[mi] = (ssb, rv)

                def stage_b(mi):
                    # normalize: single DVE tensor_scalar (f16 in/out packs to
                    # 4x mode; the fp32 per-partition scalar is allowed). The
                    # last chunk is on the kernel tail, so split it across
                    # DVE+ACT to halve the chain latency there.
                    ssb, rv = ssbs.pop(mi)
                    osb = outs.tile([P, N], f16, tag="osb")
                    if mi == MC - 1:
                        nc.vector.tensor_scalar_mul(
                            osb[:, : N // 2], ssb[:, : N // 2], rv
                        )
                        nc.scalar.mul(osb[:, N // 2 :], ssb[:, N // 2 :], rv)
                    else:
                        nc.vector.tensor_scalar_mul(osb[:], ssb[:], rv)
                    osbs[mi] = osb

                pair_strip = [None]

                def stage_c_half(mi, chalf):
                    # Two adjacent m-chunks share one output strip so the HBM
                    # write has 512B-contiguous runs (fp16, 256 m per row).
                    osb = osbs[mi]
                    half = mi % 2
                    if half == 0 and chalf == 0:
                        pair_strip[0] = otsp.tile(
                            [P, MC, 2 * P], f16, tag="ots", name="ots"
                        )
                    ots = pair_strip[0]
                    for g in range(2 * chalf, 2 * chalf + 2):
                        pst = psum_t.tile([P, 4 * P], f16, tag="pst")
                        for k in range(4):
                            j = 4 * g + k
                            nc.tensor.transpose(
                                pst[:, k * P : (k + 1) * P],
                                osb[:, j * P : (j + 1) * P],
                                ident16,
                            )
                        nc.vector.tensor_copy(
                            out=ots[:, 4 * g : 4 * g + 4, half * P : (half + 1) * P],
                            in_=pst.rearrange("p (k n) -> p k n", k=4),
                        )
                    if half == 1:
                        jh = MC // 2
                        cols = slice((mi - 1) * P, (mi + 1) * P)
                        if mi == MC - 1:
                            # tail: ship each j-quarter as soon as its
                            # transposes land
                            for g in range(2 * chalf, 2 * chalf + 2):
                                nc.sync.dma_start(
                                    O_r[:, 4 * g : 4 * g + 4, cols],
                                    ots[:, 4 * g : 4 * g + 4, :],
                                )
                        elif chalf == 0:
                            nc.sync.dma_start(O_r[:, :jh, cols], ots[:, :jh, :])
                        else:
                            nc.sync.dma_start(O_r[:, jh:, cols], ots[:, jh:, :])
                    if chalf == 1:
                        osbs.pop(mi)

                for mi in range(MC if _BUILD_PHASES[0] == 0 else 0):
                    if mi >= 1:
                        stage_c_half(mi - 1, 0)
                    stage_a_half(mi, 0)
                    stage_a_half(mi, 1)
                    stage_a_fin(mi)
                    if mi < MC - 1:
                        stage_b(mi)
                    if mi >= 1:
                        stage_c_half(mi - 1, 1)
                def stage_tail():
                    # last chunk fully quartered: normalize quarter (DVE/ACT
                    # alternating) -> transpose quarter -> ship quarter, so the
                    # exposed tail chain is one quarter deep, not one chunk
                    mi = MC - 1
                    ssb, rv = ssbs.pop(mi)
                    osb = outs.tile([P, N], f16, tag="osb", name="osb_t")
                    ots = pair_strip[0]
                    cols = slice((mi - 1) * P, (mi + 1) * P)
                    for q in range(4):
                        seg = slice(q * NB, (q + 1) * NB)
                        if q % 2 == 0:
                            nc.vector.tensor_scalar_mul(
                                osb[:, seg], ssb[:, seg], rv
                            )
                        else:
                            nc.scalar.mul(osb[:, seg], ssb[:, seg], rv)
                        pst = psum_t.tile([P, 4 * P], f16, tag="pst")
                        for k in range(4):
                            j = 4 * q + k
                            nc.tensor.transpose(
                                pst[:, k * P : (k + 1) * P],
                                osb[:, j * P : (j + 1) * P],
                                ident16,
                            )
                        nc.vector.tensor_copy(
                            out=ots[:, 4 * q : 4 * q + 4, P : 2 * P],
                            in_=pst.rearrange("p (k n) -> p k n", k=4),
                        )
                        nc.sync.dma_start(
                            O_r[:, 4 * q : 4 * q + 4, cols],
                            ots[:, 4 * q : 4 * q + 4, :],
                        )

                if _BUILD_PHASES[0] == 0:
                    stage_tail()

            for _rep in range(repeat):
                emit_once()

    _split_multi_waits(nc)
    return nc


def _get_core(repeat=1):
    """Build (once) the Bass module and its I/O metadata."""
    if ("core", repeat) in _CACHE:
        return _CACHE[("core", repeat)]

    import jax

    import concourse.mybir as mybir
    from concourse import bass2jax

    nc = _build_nc(repeat)
    bass2jax.install_neuronx_cc_hook()

    partition_name = (
        nc.partition_id_tensor.name if nc.partition_id_tensor else None
    )

    in_names = []
    out_names = []
    out_avals = []
    for alloc in nc.m.functions[0].allocations:
        if not isinstance(alloc, mybir.MemoryLocationSet):
            continue
        name = alloc.memorylocations[0].name
        if alloc.kind == "ExternalInput":
            if name != partition_name:
                in_names.append(name)
        elif alloc.kind == "ExternalOutput":
            out_names.append(name)
            out_avals.append(
                jax.core.ShapedArray(
                    tuple(alloc.tensor_shape), mybir.dt.np(alloc.dtype)
                )
            )
    in_names_all = list(in_names) + list(out_names)
    if partition_name is not None:
        in_names_all.append(partition_name)

    _CACHE[("core", repeat)] = (
        nc, partition_name, in_names, out_names, out_avals, in_names_all
    )
    return _CACHE[("core", repeat)]


def _bind_exec(nc, partition_name, in_names_all, out_names, out_avals, operands):
    from concourse import bass2jax

    if partition_name is not None:
        operands = operands + [bass2jax.partition_id_tensor()]
    return tuple(
        bass2jax._bass_exec_p.bind(
            *operands,
            out_avals=tuple(out_avals),
            in_names=tuple(in_names_all),
            out_names=tuple(out_names),
            lowering_input_output_aliases=(),
            sim_require_finite=True,
            sim_require_nnan=True,
            nc=nc,
        )
    )


def _shard_jit(body, n_in, n_out):
    import jax
    import numpy as _np
    from jax.sharding import Mesh, PartitionSpec
    from jax.experimental.shard_map import shard_map

    devices = jax.devices()[:B]
    mesh = Mesh(_np.asarray(devices), ("core",))
    in_specs = (PartitionSpec("core"),) * n_in
    out_specs = (PartitionSpec("core"),) * n_out
    return jax.jit(
        shard_map(
            body, mesh=mesh, in_specs=in_specs, out_specs=out_specs, check_rep=False
        ),
        keep_unused=True,
    )


def _get_runner(repeat=1):
    """Jitted SPMD runner: fn(*args) -> concatenated outputs."""
    if ("runner", repeat) in _CACHE:
        return _CACHE[("runner", repeat)]

    import jax
    import numpy as _np

    nc, partition_name, in_names, out_names, out_avals, in_names_all = _get_core(repeat)
    n_params = len(in_names)
    n_outs = len(out_avals)

    def _body(*args):
        return _bind_exec(
            nc, partition_name, in_names_all, out_names, out_avals, list(args)
        )

    fn = _shard_jit(_body, n_params + n_outs, n_outs)

    def pack(in_maps):
        concat_in = [
            _np.concatenate([_np.asarray(m[name]) for m in in_maps], axis=0)
            for name in in_names
        ]
        concat_zero = [
            _np.zeros((B * a.shape[0], *a.shape[1:]), a.dtype) for a in out_avals
        ]
        return [jax.device_put(a) for a in concat_in + concat_zero]

    _CACHE[("runner", repeat)] = (fn, pack, out_names, out_avals)
    return _CACHE[("runner", repeat)]


def kernel(E, W1, W2):
    E = np.ascontiguousarray(np.asarray(E), dtype=np.float32)
    W1 = np.asarray(W1, dtype=np.float32)
    W2 = np.asarray(W2, dtype=np.float32)
    # Fold the two projections: scores = E (W1 W2^T) E^T. Done in float64 on
    # host for accuracy; negligible cost (512^3 FLOPs).
    Mw = (W1.astype(np.float64) @ W2.astype(np.float64).T).astype(np.float32)

    fn, pack, out_names, out_avals = _get_runner()
    in_maps = [{"E": E[b], "M": Mw} for b in range(B)]
    args = pack(in_maps)
    outs = fn(*args)
    o = np.asarray(outs[0])  # [8*N, N] fp16
    return o.reshape(B, N, N).astype(np.float32)


if __name__ == "__main__":
    rng = np.random.default_rng(0)
    E = rng.standard_normal((B, N, D), dtype=np.float32)
    W1 = rng.standard_normal((D, D), dtype=np.float32) * (2.0 / (D + D)) ** 0.5
    W2 = rng.standard_normal((D, D), dtype=np.float32) * (2.0 / (D + D)) ** 0.5
    out = kernel(E=E, W1=W1, W2=W2)
    print(out.shape, out.dtype, out.sum())



# revision 7
# speedup vs baseline: 1.1503x; 1.1503x over previous
"""Trainium2 Bass kernel for nn_Attn_52432960749709.

Computes, for E:[B,N,D], W1/W2:[D,D]:
    q = E @ W1 ; k = E @ W2
    scores = (q @ k^T) / sqrt(D)          # per batch, [N, N]
    out = softmax(scores, axis=1)         # normalize over rows n, per column m

Strategy (data parallel over B across 8 NeuronCores, one batch element per
core; the small DxD weights are folded on the host into M = W1 @ W2^T and
replicated):

    scores = E M E^T / sqrt(D)
    Per core (one NeuronCore per batch element):
      head    14 PE warmup transposes span the ~3us clock-ramp window while
              the first loads land (E pair 0 as two single tiles, M as four
              per-chunk DMA+f32r-round pairs, each matched to its first
              consumer's order)
      E^T     PE transposes (fp32), pipelined per E-pair with the
              G^T = M E^T f32r matmuls one pair behind
      s^T     [128 m, 512 n] f32r matmuls; ACT exp(scale*s) -> fp16 strip
              with accum_out producing Z per partition; DVE: 1/Z, then one
              4x-packed tensor_scalar normalize per chunk
      out     the normalized strip [128 m, 2048 n] IS a row block of
              out^T = softmax(s^T) along its free axis, so it DMAs straight
              to HBM as 4KB-contiguous rows of O^T — no transpose-back
      tail    the last chunk runs quarter-granular: normalize quarter
              (DVE/ACT alternating) -> quarter DMA
    The device emits O^T per batch element; the host transposes the last two
    axes while upcasting fp16 -> fp32 during the unshard (pure layout work).
"""

import math

import numpy as np

B, N, D = 8, 2048, 512
P = 128
DC = D // P  # 4 contraction chunks
NB = 512  # matmul moving free dim
NBS = N // NB  # 4 n-blocks per row strip
MC = N // P  # 16 m-chunks per core

_CACHE: dict = {}

# debug: limit build to first K phases (0=all): 1=loads+ET, 2=+GT
_BUILD_PHASES = [0]


def _patch_tile_drain():
    """This walrus build rejects >1 extra sem wait on one TPB_CTRL
    instruction, so split the end-of-kernel drain's wait set across chained
    SP NOPs (same engine, so program order preserves barrier semantics)."""
    import concourse.tile as tile
    from concourse.vector_clock import ScopedClock

    if getattr(tile.TileContext, "_drain_split_patched", False):
        return

    max_waits = 1

    def _drain_and_barrier_split(self, tick_clock, wait_clock):
        nc = self.nc
        drain_inst = nc.sync.drain()
        wait_clock.add_sem_waits(
            drain_inst.ins, ScopedClock({None: tick_clock.global_clock})
        )
        si = drain_inst.ins.sync_info
        waits = list(si.on_wait or []) if si is not None else []
        if len(waits) > max_waits:
            si.on_wait = waits[:max_waits]
            rest = waits[max_waits:]
            while rest:
                nop = nc.sync.nop(nofuse=True, hint="drain_wait_split")
                chunk, rest = rest[:max_waits], rest[max_waits:]
                nsi = nop.ins.sync_info
                if nsi is None:
                    import bass_rust

                    nop.ins.sync_info = bass_rust.SyncInfo(
                        on_wait=chunk, on_update=[]
                    )
                else:
                    nsi.on_wait = chunk

        nc.all_engine_barrier()
        assert self.sems is not None
        popped = nc._tile_sem_poison_stack.pop()
        assert popped is self._sem_poison
        nc.clear_and_free_semaphores(list(self.sems.allocated().values()))
        nc.all_engine_barrier()

    tile.TileContext._drain_and_barrier = _drain_and_barrier_split
    tile.TileContext._drain_split_patched = True


def _split_multi_waits(nc):
    """This walrus build supports only one sem-wait command per instruction.
    Hoist extra waits onto same-engine NOPs inserted just before the
    instruction (engines execute in order, so semantics are preserved)."""
    import bass_rust
    import concourse.mybir as mybir

    ctr = 0
    for fn in nc.m.functions:
        for blk in fn.blocks:
            insts = blk.instructions
            out = []
            changed = False
            for inst in insts:
                si = inst.sync_info
                waits = list(si.on_wait) if (si is not None and si.on_wait) else []
                if len(waits) > 1:
                    changed = True
                    for w in waits[:-1]:
                        ctr += 1
                        nop = mybir.InstNoOp(name=f"I-waitsplit-{ctr}")
                        nop.engine = inst.engine
                        nop.sync_info = bass_rust.SyncInfo(
                            on_wait=[w], on_update=[]
                        )
                        nc.register_instruction(nop)
                        out.append(nop)
                    si.on_wait = waits[-1:]
                out.append(inst)
            if changed:
                blk.instructions = out


def _build_nc(repeat=1):
    import concourse.bass as bass
    import concourse.mybir as mybir
    import concourse.tile as tile
    from concourse.masks import make_identity

    _patch_tile_drain()

    dt = mybir.dt
    f32, f32r, f16 = dt.float32, dt.float32r, dt.float16
    Exp = mybir.ActivationFunctionType.Exp
    X = mybir.AxisListType.X

    scale = 1.0 / math.sqrt(float(D))

    nc = bass.Bass()
    E_d = nc.dram_tensor("E", [N, D], f32, kind="ExternalInput")
    M_d = nc.dram_tensor("M", [D, D], f32, kind="ExternalInput")
    # Holds out^T for this batch element: O[m, n] = softmax(s)[n, m].
    O_d = nc.dram_tensor("O", [N, N], f16, kind="ExternalOutput")

    with tile.TileContext(nc) as tc:
        with (
            tc.tile_pool(name="persist", bufs=1) as persist,
            tc.tile_pool(name="ein", bufs=6) as ein,
            tc.tile_pool(name="exps", bufs=4) as exps,
            tc.tile_pool(name="outs", bufs=3) as outs,
            tc.tile_pool(name="small", bufs=8) as small,
            tc.tile_pool(name="psum_s", bufs=3, space="PSUM") as psum_s,
        ):
            ident32 = persist.tile([P, P], f32, tag="id32")
            make_identity(nc, ident32)

            # PE warmup: keep the PE busy while the first E tiles stream in,
            # so the HAM clock gate is released before real work arrives.
            warm = psum_s.tile([P, 2 * NB], f32, tag="ps", name="warm")
            for k in range(14):
                nc.tensor.transpose(
                    warm[:, (k % 4) * P : (k % 4 + 1) * P], ident32, ident32
                )

            # f32r: fp32 storage pre-rounded for single-pass PE matmul; the
            # producing copy instruction performs the rounding.
            ET = persist.tile([P, DC, N], f32r, tag="ET")  # E^T  [d, n]
            GT = persist.tile([P, DC, N], f32r, tag="GT")  # G^T  [d', n]
            Mraw = persist.tile([P, DC, D], f32, tag="Mraw")
            Msb = persist.tile([P, DC, D], f32r, tag="M")  # M    [d, d']

            m_loaded = [False]

            def load_M():
                # per-chunk loads + rounds so gt_pair(0)'s dc=0 weights are
                # ready ~4us earlier than with one monolithic 1MB load
                if m_loaded[0]:
                    return
                m_loaded[0] = True
                for c in range(DC):
                    nc.sync.dma_start(Mraw[:, c, :], M_d[c * P : (c + 1) * P, :])
                    nc.vector.tensor_copy(out=Msb[:, c, :], in_=Mraw[:, c, :])

            def emit_once():
                # ---- E^T (PE transposes) with G^T = M E^T interleaved ----
                # E tiles processed in pairs -> 256 E^T columns at a time; the
                # pair's GT columns (all d') are computed one pair behind the
                # transposes so PE never waits on the DVE copies.
                W2C = 2 * P  # 256 columns per pair

                egroups = {}

                def load_group(g):
                    eg = ein.tile([P, 2, D], f32, tag="eg", name="eg")
                    nc.sync.dma_start(
                        eg,
                        E_d[g * 2 * P : (g + 1) * 2 * P, :].rearrange(
                            "(o p) d -> p o d", p=P
                        ),
                    )
                    egroups[g] = eg

                def load_single(i):
                    e1 = ein.tile([P, D], f32, tag="eg1", name="e1")
                    nc.sync.dma_start(e1, E_d[i * P : (i + 1) * P, :])
                    return e1

                def et_pair(pi):
                    if pi == 0:
                        # first pair as two half-size loads so the first
                        # transposes start ~0.7us earlier; M queues behind them
                        singles = [load_single(0), load_single(1)]
                        load_M()
                    for gg in range(max(pi, 1), min(pi + 3, MC // 2)):
                        if gg not in egroups:
                            load_group(gg)
                    ps = psum_s.tile([P, 2 * NB], f32, tag="ps", name="ps_et")
                    for half in range(2):
                        o = half
                        src = (
                            singles[half][:, :]
                            if pi == 0
                            else egroups[pi][:, o, :]
                        )
                        for j in range(DC):
                            nc.tensor.transpose(
                                ps[:, j * W2C + half * P : j * W2C + (half + 1) * P],
                                src[:, j * P : (j + 1) * P],
                                ident32,
                            )
                    nc.scalar.copy(
                        out=ET[:, :, pi * W2C : (pi + 1) * W2C],
                        in_=ps.rearrange("p (c n) -> p c n", c=DC),
                    )

                def gt_pair(pi):
                    ps = psum_s.tile([P, 2 * NB], f32, tag="ps", name="ps_gt")
                    for dpc in range(DC):
                        for dc in range(DC):
                            nc.tensor.matmul(
                                ps[:, dpc * W2C : (dpc + 1) * W2C],
                                lhsT=Msb[:, dc, dpc * P : (dpc + 1) * P],
                                rhs=ET[:, dc, pi * W2C : (pi + 1) * W2C],
                                start=(dc == 0),
                                stop=(dc == DC - 1),
                            )
                    nc.vector.tensor_copy(
                        out=GT[:, :, pi * W2C : (pi + 1) * W2C],
                        in_=ps.rearrange("p (c n) -> p c n", c=DC),
                    )

                NPAIR = MC // 2
                if _BUILD_PHASES[0] == 1:
                    for pi in range(NPAIR):
                        et_pair(pi)
                else:
                    for pi in range(NPAIR):
                        et_pair(pi)
                        if pi >= 1:
                            gt_pair(pi - 1)
                    gt_pair(NPAIR - 1)

                # ---- main loop over m-chunks, software-pipelined ----
                # stage a(mi): scores matmuls + exp pass1
                # stage b(mi): 1/Z + normalize pass2 + row-block DMA
                ssbs: dict = {}
                apart: dict = {}

                def stage_a_half(mi, h):
                    if mi not in apart:
                        ssb = exps.tile([P, N], f16, tag="ssb", name="ssb")
                        zp = small.tile([P, 2], f32, tag="zp", name="zp")
                        apart[mi] = (ssb, zp)
                    ssb, zp = apart[mi]
                    ps = psum_s.tile([P, 2 * NB], f32, tag="ps", name="ps_a")
                    for q in range(2):
                        nb = 2 * h + q
                        for dc in range(DC):
                            nc.tensor.matmul(
                                ps[:, q * NB : (q + 1) * NB],
                                lhsT=ET[:, dc, mi * P : (mi + 1) * P],
                                rhs=GT[:, dc, nb * NB : (nb + 1) * NB],
                                start=(dc == 0),
                                stop=(dc == DC - 1),
                            )
                    nc.scalar.activation(
                        ssb[:, h * 2 * NB : (h + 1) * 2 * NB],
                        ps,
                        Exp,
                        scale=scale,
                        accum_out=zp[:, h : h + 1],
                    )

                def stage_a_fin(mi):
                    ssb, zp = apart.pop(mi)
                    zs = small.tile([P, 1], f32, tag="zs")
                    nc.vector.reduce_sum(zs, zp, axis=X)
                    rv = small.tile([P, 1], f32, tag="rv")
                    nc.vector.reciprocal(rv, zs)
                    ssbs[mi] = (ssb, rv)

                def stage_b(mi):
                    # normalize: single DVE tensor_scalar (f16 in/out packs to
                    # 4x mode; the fp32 per-partition scalar is allowed), then
                    # ship the strip as 16 4KB-contiguous rows of O^T.
                    ssb, rv = ssbs.pop(mi)
                    osb = outs.tile([P, N], f16, tag="osb")
                    nc.vector.tensor_scalar_mul(osb[:], ssb[:], rv)
                    nc.sync.dma_start(O_d[mi * P : (mi + 1) * P, :], osb)

                for mi in range(MC if _BUILD_PHASES[0] == 0 else 0):
                    stage_a_half(mi, 0)
                    stage_a_half(mi, 1)
                    stage_a_fin(mi)
                    if mi < MC - 1:
                        stage_b(mi)

                def stage_tail():
                    # last chunk quartered: normalize quarter (DVE/ACT
                    # alternating) -> quarter DMA, so the exposed tail chain
                    # is one quarter deep, not one chunk
                    mi = MC - 1
                    ssb, rv = ssbs.pop(mi)
                    osb = outs.tile([P, N], f16, tag="osb", name="osb_t")
                    for q in range(4):
                        seg = slice(q * NB, (q + 1) * NB)
                        if q % 2 == 0:
                            nc.vector.tensor_scalar_mul(
                                osb[:, seg], ssb[:, seg], rv
                            )
                        else:
                            nc.scalar.mul(osb[:, seg], ssb[:, seg], rv)
                        nc.sync.dma_start(
                            O_d[mi * P : (mi + 1) * P, seg], osb[:, seg]
                        )

                if _BUILD_PHASES[0] == 0:
                    stage_tail()

            for _rep in range(repeat):
                emit_once()

    _split_multi_waits(nc)
    return nc


def _get_core(repeat=1):
    """Build (once) the Bass module and its I/O metadata."""
    if ("core", repeat) in _CACHE:
        return _CACHE[("core", repeat)]

    import jax

    import concourse.mybir as mybir
    from concourse import bass2jax

    nc = _build_nc(repeat)
    bass2jax.install_neuronx_cc_hook()

    partition_name = (
        nc.partition_id_tensor.name if nc.partition_id_tensor else None
    )

    in_names = []
    out_names = []
    out_avals = []
    for alloc in nc.m.functions[0].allocations:
        if not isinstance(alloc, mybir.MemoryLocationSet):
            continue
        name = alloc.memorylocations[0].name
        if alloc.kind == "ExternalInput":
            if name != partition_name:
                in_names.append(name)
        elif alloc.kind == "ExternalOutput":
            out_names.append(name)
            out_avals.append(
                jax.core.ShapedArray(
                    tuple(alloc.tensor_shape), mybir.dt.np(alloc.dtype)
                )
            )
    in_names_all = list(in_names) + list(out_names)
    if partition_name is not None:
        in_names_all.append(partition_name)

    _CACHE[("core", repeat)] = (
        nc, partition_name, in_names, out_names, out_avals, in_names_all
    )
    return _CACHE[("core", repeat)]


def _bind_exec(nc, partition_name, in_names_all, out_names, out_avals, operands):
    from concourse import bass2jax

    if partition_name is not None:
        operands = operands + [bass2jax.partition_id_tensor()]
    return tuple(
        bass2jax._bass_exec_p.bind(
            *operands,
            out_avals=tuple(out_avals),
            in_names=tuple(in_names_all),
            out_names=tuple(out_names),
            lowering_input_output_aliases=(),
            sim_require_finite=True,
            sim_require_nnan=True,
            nc=nc,
        )
    )


def _shard_jit(body, n_in, n_out):
    import jax
    import numpy as _np
    from jax.sharding import Mesh, PartitionSpec
    from jax.experimental.shard_map import shard_map

    devices = jax.devices()[:B]
    mesh = Mesh(_np.asarray(devices), ("core",))
    in_specs = (PartitionSpec("core"),) * n_in
    out_specs = (PartitionSpec("core"),) * n_out
    return jax.jit(
        shard_map(
            body, mesh=mesh, in_specs=in_specs, out_specs=out_specs, check_rep=False
        ),
        keep_unused=True,
    )


def _get_runner(repeat=1):
    """Jitted SPMD runner: fn(*args) -> concatenated outputs."""
    if ("runner", repeat) in _CACHE:
        return _CACHE[("runner", repeat)]

    import jax
    import numpy as _np

    nc, partition_name, in_names, out_names, out_avals, in_names_all = _get_core(repeat)
    n_params = len(in_names)
    n_outs = len(out_avals)

    def _body(*args):
        return _bind_exec(
            nc, partition_name, in_names_all, out_names, out_avals, list(args)
        )

    fn = _shard_jit(_body, n_params + n_outs, n_outs)

    def pack(in_maps):
        concat_in = [
            _np.concatenate([_np.asarray(m[name]) for m in in_maps], axis=0)
            for name in in_names
        ]
        concat_zero = [
            _np.zeros((B * a.shape[0], *a.shape[1:]), a.dtype) for a in out_avals
        ]
        return [jax.device_put(a) for a in concat_in + concat_zero]

    _CACHE[("runner", repeat)] = (fn, pack, out_names, out_avals)
    return _CACHE[("runner", repeat)]


def kernel(E, W1, W2):
    E = np.ascontiguousarray(np.asarray(E), dtype=np.float32)
    W1 = np.asarray(W1, dtype=np.float32)
    W2 = np.asarray(W2, dtype=np.float32)
    # Fold the two projections: scores = E (W1 W2^T) E^T. Done in float64 on
    # host for accuracy; negligible cost (512^3 FLOPs).
    Mw = (W1.astype(np.float64) @ W2.astype(np.float64).T).astype(np.float32)

    fn, pack, out_names, out_avals = _get_runner()
    in_maps = [{"E": E[b], "M": Mw} for b in range(B)]
    args = pack(in_maps)
    outs = fn(*args)
    o = np.asarray(outs[0])  # [8*N, N] fp16; each [N, N] block is out[b]^T
    return o.reshape(B, N, N).transpose(0, 2, 1).astype(np.float32)


if __name__ == "__main__":
    rng = np.random.default_rng(0)
    E = rng.standard_normal((B, N, D), dtype=np.float32)
    W1 = rng.standard_normal((D, D), dtype=np.float32) * (2.0 / (D + D)) ** 0.5
    W2 = rng.standard_normal((D, D), dtype=np.float32) * (2.0 / (D + D)) ** 0.5
    out = kernel(E=E, W1=W1, W2=W2)
    print(out.shape, out.dtype, out.sum())



# revision 25
# speedup vs baseline: 1.2675x; 1.1019x over previous
"""Trainium2 Bass kernel for nn_Attn_52432960749709.

Computes, for E:[B,N,D], W1/W2:[D,D]:
    q = E @ W1 ; k = E @ W2
    scores = (q @ k^T) / sqrt(D)          # per batch, [N, N]
    out = softmax(scores, axis=1)         # normalize over rows n, per column m

Strategy (data parallel over B across 8 NeuronCores, one batch element per
core; the small DxD weights are folded on the host into M = W1 @ W2^T and
replicated):

    scores = E M E^T / sqrt(D)
    Per core (one NeuronCore per batch element):
      head    14 PE warmup transposes span the ~3us clock-ramp window while
              the first loads land (E pair 0 as two single tiles, M as four
              per-chunk DMA+f32r-round pairs, each matched to its first
              consumer's order)
      E^T     PE transposes (fp32), pipelined per E-pair with the
              G^T = M E^T f32r matmuls one pair behind
      s^T     [128 m, 512 n] f32r matmuls; ACT exp(scale*s) -> fp16 strip
              with accum_out producing Z per partition; DVE: 1/Z, then one
              4x-packed tensor_scalar normalize per chunk
      out     the normalized strip [128 m, 2048 n] IS a row block of
              out^T = softmax(s^T) along its free axis, so it DMAs straight
              to HBM as 4KB-contiguous rows of O^T — no transpose-back
      tail    the last chunk runs quarter-granular: normalize quarter
              (DVE/ACT alternating) -> quarter DMA
    The device emits O^T per batch element; the host transposes the last two
    axes while upcasting fp16 -> fp32 during the unshard (pure layout work).
"""

import math

import numpy as np

B, N, D = 8, 2048, 512
P = 128
DC = D // P  # 4 contraction chunks
NB = 512  # matmul moving free dim
NBS = N // NB  # 4 n-blocks per row strip
MC = N // P  # 16 m-chunks per core

_CACHE: dict = {}

# debug: limit build to first K phases (0=all): 1=loads+ET, 2=+GT
_BUILD_PHASES = [0]


def _patch_tile_drain():
    """This walrus build rejects >1 extra sem wait on one TPB_CTRL
    instruction, so split the end-of-kernel drain's wait set across chained
    SP NOPs (same engine, so program order preserves barrier semantics)."""
    import concourse.tile as tile
    from concourse.vector_clock import ScopedClock

    if getattr(tile.TileContext, "_drain_split_patched", False):
        return

    max_waits = 1

    def _drain_and_barrier_split(self, tick_clock, wait_clock):
        nc = self.nc
        drain_inst = nc.sync.drain()
        wait_clock.add_sem_waits(
            drain_inst.ins, ScopedClock({None: tick_clock.global_clock})
        )
        si = drain_inst.ins.sync_info
        waits = list(si.on_wait or []) if si is not None else []
        if len(waits) > max_waits:
            si.on_wait = waits[:max_waits]
            rest = waits[max_waits:]
            while rest:
                nop = nc.sync.nop(nofuse=True, hint="drain_wait_split")
                chunk, rest = rest[:max_waits], rest[max_waits:]
                nsi = nop.ins.sync_info
                if nsi is None:
                    import bass_rust

                    nop.ins.sync_info = bass_rust.SyncInfo(
                        on_wait=chunk, on_update=[]
                    )
                else:
                    nsi.on_wait = chunk

        nc.all_engine_barrier()
        assert self.sems is not None
        popped = nc._tile_sem_poison_stack.pop()
        assert popped is self._sem_poison
        nc.clear_and_free_semaphores(list(self.sems.allocated().values()))
        nc.all_engine_barrier()

    tile.TileContext._drain_and_barrier = _drain_and_barrier_split
    tile.TileContext._drain_split_patched = True


def _split_multi_waits(nc):
    """This walrus build supports only one sem-wait command per instruction.
    Hoist extra waits onto same-engine NOPs inserted just before the
    instruction (engines execute in order, so semantics are preserved)."""
    import bass_rust
    import concourse.mybir as mybir

    ctr = 0
    for fn in nc.m.functions:
        for blk in fn.blocks:
            insts = blk.instructions
            out = []
            changed = False
            for inst in insts:
                si = inst.sync_info
                waits = list(si.on_wait) if (si is not None and si.on_wait) else []
                if len(waits) > 1:
                    changed = True
                    for w in waits[:-1]:
                        ctr += 1
                        nop = mybir.InstNoOp(name=f"I-waitsplit-{ctr}")
                        nop.engine = inst.engine
                        nop.sync_info = bass_rust.SyncInfo(
                            on_wait=[w], on_update=[]
                        )
                        nc.register_instruction(nop)
                        out.append(nop)
                    si.on_wait = waits[-1:]
                out.append(inst)
            if changed:
                blk.instructions = out


def _build_nc(repeat=1):
    import concourse.bass as bass
    import concourse.mybir as mybir
    import concourse.tile as tile
    from concourse.masks import make_identity

    _patch_tile_drain()

    dt = mybir.dt
    f32, f16, bf16 = dt.float32, dt.float16, dt.bfloat16
    Exp = mybir.ActivationFunctionType.Exp
    X = mybir.AxisListType.X

    scale = 1.0 / math.sqrt(float(D))

    nc = bass.Bass()
    # Host ships E and M pre-cast to bf16 (XBAR transpose DMA needs 2-byte
    # dtype; matmuls run bf16 anyway). Halves the load traffic too.
    E_d = nc.dram_tensor("E", [N, D], bf16, kind="ExternalInput")
    M_d = nc.dram_tensor("M", [D, D], bf16, kind="ExternalInput")
    # Holds out^T for this batch element: O[m, n] = softmax(s)[n, m].
    O_d = nc.dram_tensor("O", [N, N], f16, kind="ExternalOutput")

    with tile.TileContext(nc) as tc:
        with (
            tc.tile_pool(name="persist", bufs=1) as persist,
            tc.tile_pool(name="exps", bufs=8) as exps,
            tc.tile_pool(name="outs", bufs=3) as outs,
            tc.tile_pool(name="small", bufs=8) as small,
            tc.tile_pool(name="psum_s", bufs=4, space="PSUM") as psum_s,
        ):
            ident32 = persist.tile([P, P], f32, tag="id32")
            make_identity(nc, ident32)

            # PE warmup: keep the PE busy while the first E tiles stream in,
            # so the HAM clock gate is released before real work arrives.
            # Borrows a scores-pool PSUM tile (contents are garbage; the next
            # user overwrites via a start=True matmul).
            warm = psum_s.tile([P, 2 * NB], f32, tag="ps", name="warm")
            for k in range(14):
                nc.tensor.transpose(
                    warm[:, (k % 8) * P : (k % 8 + 1) * P], ident32, ident32
                )

            # bf16 storage: single-pass PE matmuls; precision margin is ample
            # (tolerance 2e-2, measured ~2e-3).
            ET = persist.tile([P, DC, N], bf16, tag="ET")  # E^T  [d, n]
            GT = persist.tile([P, DC, N], bf16, tag="GT")  # G^T  [d', n]
            Msb = persist.tile([P, DC, D], bf16, tag="M")  # M    [d, d']

            def emit_once():
                # ---- E^T via XBAR transpose DMAs (14ns per 16x128 tile);
                # M loaded bf16 directly. GT = M E^T streams right behind the
                # DMAs: piece pc covers 512 n-columns; within a piece the
                # matmuls run dc-innermost to match DMA arrival order, and the
                # dc-chunk of M is queued just ahead of ET(piece0, dc).
                PW = NB  # 512-column ET/GT staging piece
                NPC = N // PW  # 4 pieces

                # piece 0 lands per-dc (one 128-col XBAR DMA per M chunk, in
                # GT's consumption order) so the first GT matmul starts ~3us
                # in; pieces 1-3 are one 3D XBAR DMA each (all dc at once) to
                # keep the shared HWDGE generator off the critical path.
                for dc in range(DC):
                    # M_d holds M^T; the XBAR transpose flips it back. Using
                    # one DMA flavor end-to-end matters: mixing copy and
                    # transpose DMAs on a queue serializes them on completion
                    # semaphores (+2.5us per transition).
                    nc.sync.dma_start_transpose(
                        Msb[:, dc, :], M_d[:, dc * P : (dc + 1) * P]
                    )
                    nc.sync.dma_start_transpose(
                        ET[:, dc, 0:PW],
                        E_d[0:PW, dc * P : (dc + 1) * P],
                    )
                for pc in range(1, NPC):
                    nc.sync.dma_start_transpose(
                        ET[:, :, pc * PW : (pc + 1) * PW],
                        E_d[pc * PW : (pc + 1) * PW, :],
                    )

                def gt_piece(pc):
                    for hh in range(2):  # dpc pair per PSUM tile
                        ps = psum_s.tile(
                            [P, 2 * NB], f32, tag="ps", name="ps_gt"
                        )
                        for dc in range(DC):
                            for dq in range(2):
                                dpc = 2 * hh + dq
                                nc.tensor.matmul(
                                    ps[:, dq * NB : (dq + 1) * NB],
                                    lhsT=Msb[:, dc, dpc * P : (dpc + 1) * P],
                                    rhs=ET[:, dc, pc * PW : (pc + 1) * PW],
                                    start=(dc == 0),
                                    stop=(dc == DC - 1),
                                )
                        nc.vector.tensor_copy(
                            out=GT[
                                :, 2 * hh : 2 * hh + 2, pc * PW : (pc + 1) * PW
                            ],
                            in_=ps.rearrange("p (k n) -> p k n", k=2),
                        )

                # ---- scores, software-pipelined with the GT pieces ----
                # stage q(mi, nb): one 512-wide matmul quartet; on the odd nb
                #   of each half, ACT exp(scale*s) -> fp16 + accum Z column
                # stage b(mi): 1/Z + normalize + row-block DMA
                # The first chunks run nb-quartet-granular between GT pieces
                # (a quartet only needs GT piece nb), hiding the XBAR DMA
                # cadence behind real PE work.
                ssbs: dict = {}
                apart: dict = {}
                psq: dict = {}

                def stage_q(mi, nb, qexp=False):
                    # qexp chunks exp after every quartet (4-col accum) so
                    # their PSUM drains immediately -> no pool deadlock when
                    # a chunk is left half-done across GT pieces
                    if mi not in apart:
                        ssb = exps.tile([P, N], f16, tag="ssb", name="ssb")
                        zp = small.tile(
                            [P, 4] if qexp else [P, 2],
                            f32,
                            tag="zp4" if qexp else "zp",
                            name="zp",
                        )
                        apart[mi] = (ssb, zp)
                    ssb, zp = apart[mi]
                    if qexp or nb % 2 == 0:
                        psq[mi] = psum_s.tile(
                            [P, 2 * NB], f32, tag="ps", name="ps_a"
                        )
                    ps = psq[mi]
                    half = nb % 2
                    for dc in range(DC):
                        nc.tensor.matmul(
                            ps[:, half * NB : (half + 1) * NB],
                            lhsT=ET[:, dc, mi * P : (mi + 1) * P],
                            rhs=GT[:, dc, nb * NB : (nb + 1) * NB],
                            start=(dc == 0),
                            stop=(dc == DC - 1),
                        )
                    if qexp:
                        nc.scalar.activation(
                            ssb[:, nb * NB : (nb + 1) * NB],
                            psq.pop(mi)[:, half * NB : (half + 1) * NB],
                            Exp,
                            scale=scale,
                            accum_out=zp[:, nb : nb + 1],
                        )
                    elif nb % 2 == 1:
                        h = nb // 2
                        nc.scalar.activation(
                            ssb[:, h * 2 * NB : (h + 1) * 2 * NB],
                            psq.pop(mi),
                            Exp,
                            scale=scale,
                            accum_out=zp[:, h : h + 1],
                        )

                def stage_a_half(mi, h):
                    stage_q(mi, 2 * h)
                    stage_q(mi, 2 * h + 1)

                def stage_a_fin(mi):
                    ssb, zp = apart.pop(mi)
                    zs = small.tile([P, 1], f32, tag="zs")
                    nc.vector.reduce_sum(zs, zp, axis=X)
                    rv = small.tile([P, 1], f32, tag="rv")
                    nc.vector.reciprocal(rv, zs)
                    ssbs[mi] = (ssb, rv)

                def stage_b(mi):
                    # normalize: single DVE tensor_scalar (f16 in/out packs to
                    # 4x mode; the fp32 per-partition scalar is allowed), then
                    # ship the strip as 16 4KB-contiguous rows of O^T.
                    ssb, rv = ssbs.pop(mi)
                    osb = outs.tile([P, N], f16, tag="osb")
                    nc.vector.tensor_scalar_mul(osb[:], ssb[:], rv)
                    nc.sync.dma_start(O_d[mi * P : (mi + 1) * P, :], osb)

                # chunks 0/1 run their first quartet between gt pieces 0
                # and 1, filling the ET piece-1 DMA latency bubble
                if _BUILD_PHASES[0] == 0:
                    gt_piece(0)
                    stage_q(0, 0, qexp=True)
                    stage_q(1, 0, qexp=True)
                    gt_piece(1)
                    gt_piece(2)
                    gt_piece(3)
                    for mi in range(2):
                        for nb in range(1, 4):
                            stage_q(mi, nb, qexp=True)
                        stage_a_fin(mi)
                        stage_b(mi)
                    for mi in range(2, MC - 1):
                        stage_a_half(mi, 0)
                        stage_a_half(mi, 1)
                        stage_a_fin(mi)
                        stage_b(mi)
                else:
                    for pc in range(NPC):
                        gt_piece(pc)

                def stage_tail():
                    # last chunk fully quartered: matmul quarter -> exp
                    # quarter (own accum column), then after z closes,
                    # normalize quarter (DVE/ACT alternating) -> quarter DMA,
                    # so the exposed tail chain is one quarter deep
                    mi = MC - 1
                    ssb = exps.tile([P, N], f16, tag="ssb", name="ssb_t")
                    zp4 = small.tile([P, 4], f32, tag="zp4", name="zp4")
                    for pair in range(2):
                        ps = psum_s.tile(
                            [P, 2 * NB], f32, tag="ps", name="ps_at"
                        )
                        for q2 in range(2):
                            q = 2 * pair + q2
                            for dc in range(DC):
                                nc.tensor.matmul(
                                    ps[:, q2 * NB : (q2 + 1) * NB],
                                    lhsT=ET[:, dc, mi * P : (mi + 1) * P],
                                    rhs=GT[:, dc, q * NB : (q + 1) * NB],
                                    start=(dc == 0),
                                    stop=(dc == DC - 1),
                                )
                            nc.scalar.activation(
                                ssb[:, q * NB : (q + 1) * NB],
                                ps[:, q2 * NB : (q2 + 1) * NB],
                                Exp,
                                scale=scale,
                                accum_out=zp4[:, q : q + 1],
                            )
                    zs = small.tile([P, 1], f32, tag="zs", name="zs_t")
                    nc.vector.reduce_sum(zs, zp4, axis=X)
                    rv = small.tile([P, 1], f32, tag="rv", name="rv_t")
                    nc.vector.reciprocal(rv, zs)
                    osb = outs.tile([P, N], f16, tag="osb", name="osb_t")
                    for q in range(4):
                        seg = slice(q * NB, (q + 1) * NB)
                        if q % 2 == 0:
                            nc.vector.tensor_scalar_mul(
                                osb[:, seg], ssb[:, seg], rv
                            )
                        else:
                            nc.scalar.mul(osb[:, seg], ssb[:, seg], rv)
                        nc.sync.dma_start(
                            O_d[mi * P : (mi + 1) * P, seg], osb[:, seg]
                        )

                if _BUILD_PHASES[0] == 0:
                    stage_tail()

            for _rep in range(repeat):
                emit_once()

    _split_multi_waits(nc)
    return nc


def _get_core(repeat=1):
    """Build (once) the Bass module and its I/O metadata."""
    if ("core", repeat) in _CACHE:
        return _CACHE[("core", repeat)]

    import jax

    import concourse.mybir as mybir
    from concourse import bass2jax

    nc = _build_nc(repeat)
    bass2jax.install_neuronx_cc_hook()

    partition_name = (
        nc.partition_id_tensor.name if nc.partition_id_tensor else None
    )

    in_names = []
    out_names = []
    out_avals = []
    for alloc in nc.m.functions[0].allocations:
        if not isinstance(alloc, mybir.MemoryLocationSet):
            continue
        name = alloc.memorylocations[0].name
        if alloc.kind == "ExternalInput":
            if name != partition_name:
                in_names.append(name)
        elif alloc.kind == "ExternalOutput":
            out_names.append(name)
            out_avals.append(
                jax.core.ShapedArray(
                    tuple(alloc.tensor_shape), mybir.dt.np(alloc.dtype)
                )
            )
    in_names_all = list(in_names) + list(out_names)
    if partition_name is not None:
        in_names_all.append(partition_name)

    _CACHE[("core", repeat)] = (
        nc, partition_name, in_names, out_names, out_avals, in_names_all
    )
    return _CACHE[("core", repeat)]


def _bind_exec(nc, partition_name, in_names_all, out_names, out_avals, operands):
    from concourse import bass2jax

    if partition_name is not None:
        operands = operands + [bass2jax.partition_id_tensor()]
    return tuple(
        bass2jax._bass_exec_p.bind(
            *operands,
            out_avals=tuple(out_avals),
            in_names=tuple(in_names_all),
            out_names=tuple(out_names),
            lowering_input_output_aliases=(),
            sim_require_finite=True,
            sim_require_nnan=True,
            nc=nc,
        )
    )


def _shard_jit(body, n_in, n_out):
    import jax
    import numpy as _np
    from jax.sharding import Mesh, PartitionSpec
    from jax.experimental.shard_map import shard_map

    devices = jax.devices()[:B]
    mesh = Mesh(_np.asarray(devices), ("core",))
    in_specs = (PartitionSpec("core"),) * n_in
    out_specs = (PartitionSpec("core"),) * n_out
    return jax.jit(
        shard_map(
            body, mesh=mesh, in_specs=in_specs, out_specs=out_specs, check_rep=False
        ),
        keep_unused=True,
    )


def _get_runner(repeat=1):
    """Jitted SPMD runner: fn(*args) -> concatenated outputs."""
    if ("runner", repeat) in _CACHE:
        return _CACHE[("runner", repeat)]

    import jax
    import numpy as _np

    nc, partition_name, in_names, out_names, out_avals, in_names_all = _get_core(repeat)
    n_params = len(in_names)
    n_outs = len(out_avals)

    def _body(*args):
        return _bind_exec(
            nc, partition_name, in_names_all, out_names, out_avals, list(args)
        )

    fn = _shard_jit(_body, n_params + n_outs, n_outs)

    def pack(in_maps):
        concat_in = [
            _np.concatenate([_np.asarray(m[name]) for m in in_maps], axis=0)
            for name in in_names
        ]
        concat_zero = [
            _np.zeros((B * a.shape[0], *a.shape[1:]), a.dtype) for a in out_avals
        ]
        return [jax.device_put(a) for a in concat_in + concat_zero]

    _CACHE[("runner", repeat)] = (fn, pack, out_names, out_avals)
    return _CACHE[("runner", repeat)]


def kernel(E, W1, W2):
    import ml_dtypes

    E = np.ascontiguousarray(np.asarray(E), dtype=np.float32)
    W1 = np.asarray(W1, dtype=np.float32)
    W2 = np.asarray(W2, dtype=np.float32)
    # Fold the two projections: scores = E (W1 W2^T) E^T. Done in float64 on
    # host for accuracy; negligible cost (512^3 FLOPs).
    Mw = (W1.astype(np.float64) @ W2.astype(np.float64).T).astype(np.float32)
    # Device datapath is bf16; cast on host so XBAR transpose DMAs (2-byte
    # dtype only) can deliver E^T straight out of the load.
    Ebf = E.astype(ml_dtypes.bfloat16)
    Mbf = np.ascontiguousarray(Mw.T).astype(ml_dtypes.bfloat16)

    fn, pack, out_names, out_avals = _get_runner()
    in_maps = [{"E": Ebf[b], "M": Mbf} for b in range(B)]
    args = pack(in_maps)
    outs = fn(*args)
    o = np.asarray(outs[0])  # [8*N, N] fp16; each [N, N] block is out[b]^T
    return o.reshape(B, N, N).transpose(0, 2, 1).astype(np.float32)


if __name__ == "__main__":
    rng = np.random.default_rng(0)
    E = rng.standard_normal((B, N, D), dtype=np.float32)
    W1 = rng.standard_normal((D, D), dtype=np.float32) * (2.0 / (D + D)) ** 0.5
    W2 = rng.standard_normal((D, D), dtype=np.float32) * (2.0 / (D + D)) ** 0.5
    out = kernel(E=E, W1=W1, W2=W2)
    print(out.shape, out.dtype, out.sum())



# revision 41
# speedup vs baseline: 1.2825x; 1.0118x over previous
"""Trainium2 Bass kernel for nn_Attn_52432960749709.

Computes, for E:[B,N,D], W1/W2:[D,D]:
    q = E @ W1 ; k = E @ W2
    scores = (q @ k^T) / sqrt(D)          # per batch, [N, N]
    out = softmax(scores, axis=1)         # normalize over rows n, per column m

Strategy (data parallel over B across 8 NeuronCores, one batch element per
core; the small DxD weights are folded on the host into M = W1 @ W2^T and
replicated):

    scores = E M E^T / sqrt(D)
    Per core (one NeuronCore per batch element), all-bf16 datapath (host
    pre-casts E and ships M^T; tolerance is 2e-2, measured ~7e-3):
      head    14 PE warmup transposes (garbage data) burn the ~3us clock-ramp
              window so every real matmul runs at the full 2.4 GHz
      E^T, M  XBAR transpose DMAs deliver E^T and M straight out of HBM
              (14ns per 16x128 tile, ~12x cheaper than copy descriptors) —
              zero PE/ACT/Pool work. All loads stay on the one SP queue:
              per-dc pieces for the first two 512-column n-pieces (so GT
              streams behind arrivals), one 3D DMA for each later piece.
              HW gotchas baked in here: (a) mixing copy- and transpose-
              flavor DMAs on one queue chains them on completion semaphores
              (+2.5us each); (b) ACT-queue-issued XBAR DMAs silently corrupt
              data on hardware (sim-only feature, it seems).
      G^T     G^T = M E^T as 4 512-column pieces, dc-outermost so the 4
              matmuls per dc match the per-dc DMA arrival cadence; PSUM ->
              bf16 SBUF copies on DVE. The first quartet of scores matmuls
              for chunks 0-3 (with per-quartet exp into a 4-column accum)
              interleaves between GT pieces to cover DMA latency.
      s^T     per m-chunk [128 m, 2048 n]: 16 bf16 matmuls (512-wide — the
              PSUM-bank limit); ACT exp(scale*s) -> fp16 strip per 1024-half
              with accum_out building Z per partition; DVE: 1/Z then one
              4x-packed tensor_scalar normalize
      out     the normalized strip IS a row block of out^T = softmax(s^T)
              along its free axis, so it DMAs straight to HBM as
              4KB-contiguous rows of O^T — no transpose-back
      tail    the last chunk runs in shrinking segments (1024/768/256):
              exp per segment, and after Z closes, DVE normalizes while the
              SP and ACT queues split the three segment DMAs
    The device emits O^T per batch element; the host transposes the last two
    axes while upcasting fp16 -> fp32 during the unshard (pure layout work).
"""

import math

import numpy as np

B, N, D = 8, 2048, 512
P = 128
DC = D // P  # 4 contraction chunks
NB = 512  # matmul moving free dim
NBS = N // NB  # 4 n-blocks per row strip
MC = N // P  # 16 m-chunks per core

_CACHE: dict = {}

# debug: limit build to first K phases (0=all): 1=loads+ET, 2=+GT
_BUILD_PHASES = [0]


def _patch_tile_drain():
    """This walrus build rejects >1 extra sem wait on one TPB_CTRL
    instruction, so split the end-of-kernel drain's wait set across chained
    SP NOPs (same engine, so program order preserves barrier semantics)."""
    import concourse.tile as tile
    from concourse.vector_clock import ScopedClock

    if getattr(tile.TileContext, "_drain_split_patched", False):
        return

    max_waits = 1

    def _drain_and_barrier_split(self, tick_clock, wait_clock):
        nc = self.nc
        drain_inst = nc.sync.drain()
        wait_clock.add_sem_waits(
            drain_inst.ins, ScopedClock({None: tick_clock.global_clock})
        )
        si = drain_inst.ins.sync_info
        waits = list(si.on_wait or []) if si is not None else []
        if len(waits) > max_waits:
            si.on_wait = waits[:max_waits]
            rest = waits[max_waits:]
            while rest:
                nop = nc.sync.nop(nofuse=True, hint="drain_wait_split")
                chunk, rest = rest[:max_waits], rest[max_waits:]
                nsi = nop.ins.sync_info
                if nsi is None:
                    import bass_rust

                    nop.ins.sync_info = bass_rust.SyncInfo(
                        on_wait=chunk, on_update=[]
                    )
                else:
                    nsi.on_wait = chunk

        nc.all_engine_barrier()
        assert self.sems is not None
        popped = nc._tile_sem_poison_stack.pop()
        assert popped is self._sem_poison
        nc.clear_and_free_semaphores(list(self.sems.allocated().values()))
        nc.all_engine_barrier()

    tile.TileContext._drain_and_barrier = _drain_and_barrier_split
    tile.TileContext._drain_split_patched = True


def _split_multi_waits(nc):
    """This walrus build supports only one sem-wait command per instruction.
    Hoist extra waits onto same-engine NOPs inserted just before the
    instruction (engines execute in order, so semantics are preserved)."""
    import bass_rust
    import concourse.mybir as mybir

    ctr = 0
    for fn in nc.m.functions:
        for blk in fn.blocks:
            insts = blk.instructions
            out = []
            changed = False
            for inst in insts:
                si = inst.sync_info
                waits = list(si.on_wait) if (si is not None and si.on_wait) else []
                if len(waits) > 1:
                    changed = True
                    for w in waits[:-1]:
                        ctr += 1
                        nop = mybir.InstNoOp(name=f"I-waitsplit-{ctr}")
                        nop.engine = inst.engine
                        nop.sync_info = bass_rust.SyncInfo(
                            on_wait=[w], on_update=[]
                        )
                        nc.register_instruction(nop)
                        out.append(nop)
                    si.on_wait = waits[-1:]
                out.append(inst)
            if changed:
                blk.instructions = out


def _build_nc(repeat=1):
    import concourse.bass as bass
    import concourse.mybir as mybir
    import concourse.tile as tile
    from concourse.masks import make_identity

    _patch_tile_drain()

    dt = mybir.dt
    f32, f16, bf16 = dt.float32, dt.float16, dt.bfloat16
    Exp = mybir.ActivationFunctionType.Exp
    X = mybir.AxisListType.X

    scale = 1.0 / math.sqrt(float(D))

    nc = bass.Bass()
    # Host ships E and M pre-cast to bf16 (XBAR transpose DMA needs 2-byte
    # dtype; matmuls run bf16 anyway). Halves the load traffic too.
    E_d = nc.dram_tensor("E", [N, D], bf16, kind="ExternalInput")
    M_d = nc.dram_tensor("M", [D, D], bf16, kind="ExternalInput")
    # Holds out^T for this batch element: O[m, n] = softmax(s)[n, m].
    O_d = nc.dram_tensor("O", [N, N], f16, kind="ExternalOutput")

    with tile.TileContext(nc) as tc:
        with (
            tc.tile_pool(name="persist", bufs=1) as persist,
            tc.tile_pool(name="exps", bufs=8) as exps,
            tc.tile_pool(name="outs", bufs=3) as outs,
            tc.tile_pool(name="small", bufs=8) as small,
            tc.tile_pool(name="psum_s", bufs=4, space="PSUM") as psum_s,
        ):
            # warmup source: content irrelevant (transposes are throwaway);
            # a single Pool memset is the cheapest legal producer
            ident32 = persist.tile([P, P], f32, tag="id32")
            nc.gpsimd.memset(ident32, 0.0)

            # PE warmup: keep the PE busy while the first E tiles stream in,
            # so the HAM clock gate is released before real work arrives.
            # Borrows a scores-pool PSUM tile (contents are garbage; the next
            # user overwrites via a start=True matmul).
            warm = psum_s.tile([P, 2 * NB], f32, tag="ps", name="warm")
            for k in range(14):
                nc.tensor.transpose(
                    warm[:, (k % 8) * P : (k % 8 + 1) * P], ident32, ident32
                )

            # bf16 storage: single-pass PE matmuls; precision margin is ample
            # (tolerance 2e-2, measured ~2e-3).
            ET = persist.tile([P, DC, N], bf16, tag="ET")  # E^T  [d, n]
            GT = persist.tile([P, DC, N], bf16, tag="GT")  # G^T  [d', n]
            Msb = persist.tile([P, DC, D], bf16, tag="M")  # M    [d, d']

            def emit_once():
                # ---- E^T via XBAR transpose DMAs (14ns per 16x128 tile);
                # M loaded bf16 directly. GT = M E^T streams right behind the
                # DMAs: piece pc covers 512 n-columns; within a piece the
                # matmuls run dc-innermost to match DMA arrival order, and the
                # dc-chunk of M is queued just ahead of ET(piece0, dc).
                PW = NB  # 512-column ET/GT staging piece
                NPC = N // PW  # 4 pieces

                # All loads are XBAR transposes (M_d holds M^T and is
                # flipped back in-flight). Two rules shape this schedule:
                # (1) one DMA flavor per queue end-to-end — mixing copy and
                # transpose DMAs on a queue serializes them on completion
                # semaphores (+2.5us per transition); (2) each queue issues a
                # DMA only every ~0.65us, so the loads alternate between the
                # SP and ACT queues, in GT's consumption order, to keep the
                # DMA device streaming back-to-back.
                # per-dc front loads alternating between the SP and ACT
                # queues (each queue issues one DMA per ~0.65us), in GT's
                # consumption order: M chunk dc just ahead of ET(piece, dc)
                def ldt(dst, srcap):
                    nc.sync.dma_start_transpose(dst, srcap)

                for dc in range(DC):
                    ldt(Msb[:, dc, :], M_d[:, dc * P : (dc + 1) * P])
                    ldt(
                        ET[:, dc, 0:PW],
                        E_d[0:PW, dc * P : (dc + 1) * P],
                    )
                for dc in range(DC):
                    ldt(
                        ET[:, dc, PW : 2 * PW],
                        E_d[PW : 2 * PW, dc * P : (dc + 1) * P],
                    )
                for pc in range(2, NPC):
                    ldt(
                        ET[:, :, pc * PW : (pc + 1) * PW],
                        E_d[pc * PW : (pc + 1) * PW, :],
                    )

                def gt_piece(pc):
                    # dc outermost across BOTH psum halves: 4 matmuls per dc
                    # (~0.85us) matches the per-dc XBAR arrival cadence, so
                    # piece 0 streams without per-dc stalls
                    pss = [
                        psum_s.tile([P, 2 * NB], f32, tag="ps", name="ps_gt")
                        for _ in range(2)
                    ]
                    for dc in range(DC):  # dc-pair groups match DMA sems
                        for hh in range(2):
                            for dq in range(2):
                                dpc = 2 * hh + dq
                                nc.tensor.matmul(
                                    pss[hh][:, dq * NB : (dq + 1) * NB],
                                    lhsT=Msb[:, dc, dpc * P : (dpc + 1) * P],
                                    rhs=ET[:, dc, pc * PW : (pc + 1) * PW],
                                    start=(dc == 0),
                                    stop=(dc == DC - 1),
                                )
                    for hh, eng in ((0, nc.vector), (1, nc.vector)):
                        eng.tensor_copy(
                            out=GT[
                                :, 2 * hh : 2 * hh + 2, pc * PW : (pc + 1) * PW
                            ],
                            in_=pss[hh].rearrange("p (k n) -> p k n", k=2),
                        )

                # ---- scores, software-pipelined with the GT pieces ----
                # stage q(mi, nb): one 512-wide matmul quartet; on the odd nb
                #   of each half, ACT exp(scale*s) -> fp16 + accum Z column
                # stage b(mi): 1/Z + normalize + row-block DMA
                # The first chunks run nb-quartet-granular between GT pieces
                # (a quartet only needs GT piece nb), hiding the XBAR DMA
                # cadence behind real PE work.
                ssbs: dict = {}
                apart: dict = {}
                psq: dict = {}

                def stage_q(mi, nb, qexp=False):
                    # qexp chunks exp after every quartet (4-col accum) so
                    # their PSUM drains immediately -> no pool deadlock when
                    # a chunk is left half-done across GT pieces
                    if mi not in apart:
                        ssb = exps.tile([P, N], f16, tag="ssb", name="ssb")
                        zp = small.tile(
                            [P, 4] if qexp else [P, 2],
                            f32,
                            tag="zp4" if qexp else "zp",
                            name="zp",
                        )
                        apart[mi] = (ssb, zp)
                    ssb, zp = apart[mi]
                    if qexp or nb % 2 == 0:
                        psq[mi] = psum_s.tile(
                            [P, 2 * NB], f32, tag="ps", name="ps_a"
                        )
                    ps = psq[mi]
                    half = nb % 2
                    for dc in range(DC):
                        nc.tensor.matmul(
                            ps[:, half * NB : (half + 1) * NB],
                            lhsT=ET[:, dc, mi * P : (mi + 1) * P],
                            rhs=GT[:, dc, nb * NB : (nb + 1) * NB],
                            start=(dc == 0),
                            stop=(dc == DC - 1),
                        )
                    if qexp:
                        nc.scalar.activation(
                            ssb[:, nb * NB : (nb + 1) * NB],
                            psq.pop(mi)[:, half * NB : (half + 1) * NB],
                            Exp,
                            scale=scale,
                            accum_out=zp[:, nb : nb + 1],
                        )
                    elif nb % 2 == 1:
                        h = nb // 2
                        nc.scalar.activation(
                            ssb[:, h * 2 * NB : (h + 1) * 2 * NB],
                            psq.pop(mi),
                            Exp,
                            scale=scale,
                            accum_out=zp[:, h : h + 1],
                        )

                def stage_a_half(mi, h):
                    stage_q(mi, 2 * h)
                    stage_q(mi, 2 * h + 1)

                def stage_a_fin(mi):
                    ssb, zp = apart.pop(mi)
                    zs = small.tile([P, 1], f32, tag="zs")
                    nc.vector.reduce_sum(zs, zp, axis=X)
                    rv = small.tile([P, 1], f32, tag="rv")
                    nc.vector.reciprocal(rv, zs)
                    ssbs[mi] = (ssb, rv)

                def stage_b(mi):
                    # normalize: single DVE tensor_scalar (f16 in/out packs to
                    # 4x mode; the fp32 per-partition scalar is allowed), then
                    # ship the strip as 16 4KB-contiguous rows of O^T.
                    ssb, rv = ssbs.pop(mi)
                    osb = outs.tile([P, N], f16, tag="osb")
                    nc.vector.tensor_scalar_mul(osb[:], ssb[:], rv)
                    nc.sync.dma_start(O_d[mi * P : (mi + 1) * P, :], osb)

                # chunks 0/1 run their first quartet between gt pieces 0
                # and 1, filling the ET piece-1 DMA latency bubble
                if _BUILD_PHASES[0] == 0:
                    gt_piece(0)
                    stage_q(0, 0, qexp=True)
                    stage_q(1, 0, qexp=True)
                    stage_q(2, 0, qexp=True)
                    stage_q(3, 0, qexp=True)
                    gt_piece(1)
                    gt_piece(2)
                    gt_piece(3)
                    for mi in range(4):
                        for nb in range(1, 4):
                            stage_q(mi, nb, qexp=True)
                        stage_a_fin(mi)
                        stage_b(mi)
                    for mi in range(4, MC - 1):
                        stage_a_half(mi, 0)
                        stage_a_half(mi, 1)
                        stage_a_fin(mi)
                        stage_b(mi)
                else:
                    for pc in range(NPC):
                        gt_piece(pc)

                def stage_tail():
                    # last chunk in shrinking segments (512x3 + 256x2): each
                    # segment's matmuls -> exp (own accum column); after z
                    # closes, normalize segments alternate DVE/ACT and DMAs
                    # split across the SP and ACT queues so the exposed chain
                    # after the final matmul is one 256-col segment deep
                    mi = MC - 1
                    rows = slice(mi * P, (mi + 1) * P)
                    segs = [(0, 1024), (1024, 768), (1792, 256)]
                    ssb = exps.tile([P, N], f16, tag="ssb", name="ssb_t")
                    zp = small.tile([P, 4], f32, tag="zp5", name="zp5")
                    for si, (c0, w) in enumerate(segs):
                        ps = psum_s.tile(
                            [P, 2 * NB], f32, tag="ps", name="ps_at"
                        )
                        # matmul moving dim caps at 512 (one PSUM bank); the
                        # exp still reads the whole segment in one pass
                        for b0 in range(0, w, NB):
                            bw = min(NB, w - b0)
                            for dc in range(DC):
                                nc.tensor.matmul(
                                    ps[:, b0 : b0 + bw],
                                    lhsT=ET[:, dc, rows],
                                    rhs=GT[:, dc, c0 + b0 : c0 + b0 + bw],
                                    start=(dc == 0),
                                    stop=(dc == DC - 1),
                                )
                        nc.scalar.activation(
                            ssb[:, c0 : c0 + w],
                            ps[:, 0:w],
                            Exp,
                            scale=scale,
                            accum_out=zp[:, si : si + 1],
                        )
                    zs = small.tile([P, 1], f32, tag="zs", name="zs_t")
                    nc.vector.reduce_sum(zs, zp[:, : len(segs)], axis=X)
                    rv = small.tile([P, 1], f32, tag="rv", name="rv_t")
                    nc.vector.reciprocal(rv, zs)
                    osb = outs.tile([P, N], f16, tag="osb", name="osb_t")
                    dma_eng = [nc.sync, nc.scalar, nc.sync]
                    for si, (c0, w) in enumerate(segs):
                        seg = slice(c0, c0 + w)
                        nc.vector.tensor_scalar_mul(
                            osb[:, seg], ssb[:, seg], rv
                        )
                        dma_eng[si].dma_start(O_d[rows, seg], osb[:, seg])

                if _BUILD_PHASES[0] == 0:
                    stage_tail()

            for _rep in range(repeat):
                emit_once()

    _split_multi_waits(nc)
    return nc


def _get_core(repeat=1):
    """Build (once) the Bass module and its I/O metadata."""
    if ("core", repeat) in _CACHE:
        return _CACHE[("core", repeat)]

    import jax

    import concourse.mybir as mybir
    from concourse import bass2jax

    nc = _build_nc(repeat)
    bass2jax.install_neuronx_cc_hook()

    partition_name = (
        nc.partition_id_tensor.name if nc.partition_id_tensor else None
    )

    in_names = []
    out_names = []
    out_avals = []
    for alloc in nc.m.functions[0].allocations:
        if not isinstance(alloc, mybir.MemoryLocationSet):
            continue
        name = alloc.memorylocations[0].name
        if alloc.kind == "ExternalInput":
            if name != partition_name:
                in_names.append(name)
        elif alloc.kind == "ExternalOutput":
            out_names.append(name)
            out_avals.append(
                jax.core.ShapedArray(
                    tuple(alloc.tensor_shape), mybir.dt.np(alloc.dtype)
                )
            )
    in_names_all = list(in_names) + list(out_names)
    if partition_name is not None:
        in_names_all.append(partition_name)

    _CACHE[("core", repeat)] = (
        nc, partition_name, in_names, out_names, out_avals, in_names_all
    )
    return _CACHE[("core", repeat)]


def _bind_exec(nc, partition_name, in_names_all, out_names, out_avals, operands):
    from concourse import bass2jax

    if partition_name is not None:
        operands = operands + [bass2jax.partition_id_tensor()]
    return tuple(
        bass2jax._bass_exec_p.bind(
            *operands,
            out_avals=tuple(out_avals),
            in_names=tuple(in_names_all),
            out_names=tuple(out_names),
            lowering_input_output_aliases=(),
            sim_require_finite=True,
            sim_require_nnan=True,
            nc=nc,
        )
    )


def _shard_jit(body, n_in, n_out):
    import jax
    import numpy as _np
    from jax.sharding import Mesh, PartitionSpec
    from jax.experimental.shard_map import shard_map

    devices = jax.devices()[:B]
    mesh = Mesh(_np.asarray(devices), ("core",))
    in_specs = (PartitionSpec("core"),) * n_in
    out_specs = (PartitionSpec("core"),) * n_out
    return jax.jit(
        shard_map(
            body, mesh=mesh, in_specs=in_specs, out_specs=out_specs, check_rep=False
        ),
        keep_unused=True,
    )


def _get_runner(repeat=1):
    """Jitted SPMD runner: fn(*args) -> concatenated outputs."""
    if ("runner", repeat) in _CACHE:
        return _CACHE[("runner", repeat)]

    import jax
    import numpy as _np

    nc, partition_name, in_names, out_names, out_avals, in_names_all = _get_core(repeat)
    n_params = len(in_names)
    n_outs = len(out_avals)

    def _body(*args):
        return _bind_exec(
            nc, partition_name, in_names_all, out_names, out_avals, list(args)
        )

    fn = _shard_jit(_body, n_params + n_outs, n_outs)

    def pack(in_maps):
        concat_in = [
            _np.concatenate([_np.asarray(m[name]) for m in in_maps], axis=0)
            for name in in_names
        ]
        concat_zero = [
            _np.zeros((B * a.shape[0], *a.shape[1:]), a.dtype) for a in out_avals
        ]
        return [jax.device_put(a) for a in concat_in + concat_zero]

    _CACHE[("runner", repeat)] = (fn, pack, out_names, out_avals)
    return _CACHE[("runner", repeat)]


def kernel(E, W1, W2):
    import ml_dtypes

    E = np.ascontiguousarray(np.asarray(E), dtype=np.float32)
    W1 = np.asarray(W1, dtype=np.float32)
    W2 = np.asarray(W2, dtype=np.float32)
    # Fold the two projections: scores = E (W1 W2^T) E^T. Done in float64 on
    # host for accuracy; negligible cost (512^3 FLOPs).
    Mw = (W1.astype(np.float64) @ W2.astype(np.float64).T).astype(np.float32)
    # Device datapath is bf16; cast on host so XBAR transpose DMAs (2-byte
    # dtype only) can deliver E^T straight out of the load.
    Ebf = E.astype(ml_dtypes.bfloat16)
    Mbf = np.ascontiguousarray(Mw.T).astype(ml_dtypes.bfloat16)

    fn, pack, out_names, out_avals = _get_runner()
    in_maps = [{"E": Ebf[b], "M": Mbf} for b in range(B)]
    args = pack(in_maps)
    outs = fn(*args)
    o = np.asarray(outs[0])  # [8*N, N] fp16; each [N, N] block is out[b]^T
    return o.reshape(B, N, N).transpose(0, 2, 1).astype(np.float32, order="C")


if __name__ == "__main__":
    rng = np.random.default_rng(0)
    E = rng.standard_normal((B, N, D), dtype=np.float32)
    W1 = rng.standard_normal((D, D), dtype=np.float32) * (2.0 / (D + D)) ** 0.5
    W2 = rng.standard_normal((D, D), dtype=np.float32) * (2.0 / (D + D)) ** 0.5
    out = kernel(E=E, W1=W1, W2=W2)
    print(out.shape, out.dtype, out.sum())



# revision 49
# speedup vs baseline: 1.2840x; 1.0012x over previous
"""Trainium2 Bass kernel for nn_Attn_52432960749709.

Computes, for E:[B,N,D], W1/W2:[D,D]:
    q = E @ W1 ; k = E @ W2
    scores = (q @ k^T) / sqrt(D)          # per batch, [N, N]
    out = softmax(scores, axis=1)         # normalize over rows n, per column m

Strategy (data parallel over B across 8 NeuronCores, one batch element per
core; the small DxD weights are folded on the host into M = W1 @ W2^T and
replicated):

    scores = E M E^T / sqrt(D)
    Per core (one NeuronCore per batch element), all-bf16 datapath (host
    pre-casts E and ships M^T; tolerance is 2e-2, measured ~7e-3):
      head    14 PE warmup transposes (garbage data) burn the ~3us clock-ramp
              window so every real matmul runs at the full 2.4 GHz
      E^T, M  XBAR transpose DMAs deliver E^T and M straight out of HBM
              (14ns per 16x128 tile, ~12x cheaper than copy descriptors) —
              zero PE/ACT/Pool work. All loads stay on the one SP queue:
              per-dc pieces for the first two 512-column n-pieces (so GT
              streams behind arrivals), one 3D DMA for each later piece.
              HW gotchas baked in here: (a) mixing copy- and transpose-
              flavor DMAs on one queue chains them on completion semaphores
              (+2.5us each); (b) ACT-queue-issued XBAR DMAs silently corrupt
              data on hardware (sim-only feature, it seems).
      G^T     G^T = M E^T as 4 512-column pieces, dc-outermost so the 4
              matmuls per dc match the per-dc DMA arrival cadence; PSUM ->
              bf16 SBUF copies on DVE. The first quartet of scores matmuls
              for chunks 0-3 (with per-quartet exp into a 4-column accum)
              interleaves between GT pieces to cover DMA latency.
      s^T     per m-chunk [128 m, 2048 n]: 16 bf16 matmuls (512-wide — the
              PSUM-bank limit); ACT exp(scale*s) -> fp16 strip per 1024-half
              with accum_out building Z per partition; DVE: 1/Z then one
              4x-packed tensor_scalar normalize
      out     the normalized strip IS a row block of out^T = softmax(s^T)
              along its free axis, so it DMAs straight to HBM as
              4KB-contiguous rows of O^T — no transpose-back
      tail    the last chunk runs in segments (768/768/512): exp per
              segment, and after Z closes, DVE normalizes while the SP and
              ACT queues split the three segment DMAs
    The device emits O^T per batch element; the host transposes the last two
    axes while upcasting fp16 -> fp32 during the unshard (pure layout work).
"""

import math

import numpy as np

B, N, D = 8, 2048, 512
P = 128
DC = D // P  # 4 contraction chunks
NB = 512  # matmul moving free dim
NBS = N // NB  # 4 n-blocks per row strip
MC = N // P  # 16 m-chunks per core

_CACHE: dict = {}

# debug: limit build to first K phases (0=all): 1=loads+ET, 2=+GT
_BUILD_PHASES = [0]


def _patch_tile_drain():
    """This walrus build rejects >1 extra sem wait on one TPB_CTRL
    instruction, so split the end-of-kernel drain's wait set across chained
    SP NOPs (same engine, so program order preserves barrier semantics)."""
    import concourse.tile as tile
    from concourse.vector_clock import ScopedClock

    if getattr(tile.TileContext, "_drain_split_patched", False):
        return

    max_waits = 1

    def _drain_and_barrier_split(self, tick_clock, wait_clock):
        nc = self.nc
        drain_inst = nc.sync.drain()
        wait_clock.add_sem_waits(
            drain_inst.ins, ScopedClock({None: tick_clock.global_clock})
        )
        si = drain_inst.ins.sync_info
        waits = list(si.on_wait or []) if si is not None else []
        if len(waits) > max_waits:
            si.on_wait = waits[:max_waits]
            rest = waits[max_waits:]
            while rest:
                nop = nc.sync.nop(nofuse=True, hint="drain_wait_split")
                chunk, rest = rest[:max_waits], rest[max_waits:]
                nsi = nop.ins.sync_info
                if nsi is None:
                    import bass_rust

                    nop.ins.sync_info = bass_rust.SyncInfo(
                        on_wait=chunk, on_update=[]
                    )
                else:
                    nsi.on_wait = chunk

        nc.all_engine_barrier()
        assert self.sems is not None
        popped = nc._tile_sem_poison_stack.pop()
        assert popped is self._sem_poison
        nc.clear_and_free_semaphores(list(self.sems.allocated().values()))
        nc.all_engine_barrier()

    tile.TileContext._drain_and_barrier = _drain_and_barrier_split
    tile.TileContext._drain_split_patched = True


def _split_multi_waits(nc):
    """This walrus build supports only one sem-wait command per instruction.
    Hoist extra waits onto same-engine NOPs inserted just before the
    instruction (engines execute in order, so semantics are preserved)."""
    import bass_rust
    import concourse.mybir as mybir

    ctr = 0
    for fn in nc.m.functions:
        for blk in fn.blocks:
            insts = blk.instructions
            out = []
            changed = False
            for inst in insts:
                si = inst.sync_info
                waits = list(si.on_wait) if (si is not None and si.on_wait) else []
                if len(waits) > 1:
                    changed = True
                    for w in waits[:-1]:
                        ctr += 1
                        nop = mybir.InstNoOp(name=f"I-waitsplit-{ctr}")
                        nop.engine = inst.engine
                        nop.sync_info = bass_rust.SyncInfo(
                            on_wait=[w], on_update=[]
                        )
                        nc.register_instruction(nop)
                        out.append(nop)
                    si.on_wait = waits[-1:]
                out.append(inst)
            if changed:
                blk.instructions = out


def _build_nc(repeat=1):
    import concourse.bass as bass
    import concourse.mybir as mybir
    import concourse.tile as tile
    from concourse.masks import make_identity

    _patch_tile_drain()

    dt = mybir.dt
    f32, f16, bf16 = dt.float32, dt.float16, dt.bfloat16
    Exp = mybir.ActivationFunctionType.Exp
    X = mybir.AxisListType.X

    scale = 1.0 / math.sqrt(float(D))

    nc = bass.Bass()
    # Host ships E and M pre-cast to bf16 (XBAR transpose DMA needs 2-byte
    # dtype; matmuls run bf16 anyway). Halves the load traffic too.
    E_d = nc.dram_tensor("E", [N, D], bf16, kind="ExternalInput")
    M_d = nc.dram_tensor("M", [D, D], bf16, kind="ExternalInput")
    # Holds out^T for this batch element: O[m, n] = softmax(s)[n, m].
    O_d = nc.dram_tensor("O", [N, N], f16, kind="ExternalOutput")

    with tile.TileContext(nc) as tc:
        with (
            tc.tile_pool(name="persist", bufs=1) as persist,
            tc.tile_pool(name="exps", bufs=8) as exps,
            tc.tile_pool(name="outs", bufs=3) as outs,
            tc.tile_pool(name="small", bufs=8) as small,
            tc.tile_pool(name="psum_s", bufs=4, space="PSUM") as psum_s,
        ):
            # warmup source: content irrelevant (transposes are throwaway);
            # a single Pool memset is the cheapest legal producer
            ident32 = persist.tile([P, P], f32, tag="id32")
            nc.gpsimd.memset(ident32, 0.0)

            # PE warmup: keep the PE busy while the first E tiles stream in,
            # so the HAM clock gate is released before real work arrives.
            # Borrows a scores-pool PSUM tile (contents are garbage; the next
            # user overwrites via a start=True matmul).
            warm = psum_s.tile([P, 2 * NB], f32, tag="ps", name="warm")
            for k in range(14):
                nc.tensor.transpose(
                    warm[:, (k % 8) * P : (k % 8 + 1) * P], ident32, ident32
                )

            # bf16 storage: single-pass PE matmuls; precision margin is ample
            # (tolerance 2e-2, measured ~2e-3).
            ET = persist.tile([P, DC, N], bf16, tag="ET")  # E^T  [d, n]
            GT = persist.tile([P, DC, N], bf16, tag="GT")  # G^T  [d', n]
            Msb = persist.tile([P, DC, D], bf16, tag="M")  # M    [d, d']

            def emit_once():
                # ---- E^T via XBAR transpose DMAs (14ns per 16x128 tile);
                # M loaded bf16 directly. GT = M E^T streams right behind the
                # DMAs: piece pc covers 512 n-columns; within a piece the
                # matmuls run dc-innermost to match DMA arrival order, and the
                # dc-chunk of M is queued just ahead of ET(piece0, dc).
                PW = NB  # 512-column ET/GT staging piece
                NPC = N // PW  # 4 pieces

                # All loads are XBAR transposes (M_d holds M^T and is
                # flipped back in-flight). Two rules shape this schedule:
                # (1) one DMA flavor per queue end-to-end — mixing copy and
                # transpose DMAs on a queue serializes them on completion
                # semaphores (+2.5us per transition); (2) each queue issues a
                # DMA only every ~0.65us, so the loads alternate between the
                # SP and ACT queues, in GT's consumption order, to keep the
                # DMA device streaming back-to-back.
                # per-dc front loads alternating between the SP and ACT
                # queues (each queue issues one DMA per ~0.65us), in GT's
                # consumption order: M chunk dc just ahead of ET(piece, dc)
                def ldt(dst, srcap):
                    nc.sync.dma_start_transpose(dst, srcap)

                for dc in range(DC):
                    ldt(Msb[:, dc, :], M_d[:, dc * P : (dc + 1) * P])
                    ldt(
                        ET[:, dc, 0:PW],
                        E_d[0:PW, dc * P : (dc + 1) * P],
                    )
                for dc in range(DC):
                    ldt(
                        ET[:, dc, PW : 2 * PW],
                        E_d[PW : 2 * PW, dc * P : (dc + 1) * P],
                    )
                for pc in range(2, NPC):
                    ldt(
                        ET[:, :, pc * PW : (pc + 1) * PW],
                        E_d[pc * PW : (pc + 1) * PW, :],
                    )

                def gt_piece(pc):
                    # dc outermost across BOTH psum halves: 4 matmuls per dc
                    # (~0.85us) matches the per-dc XBAR arrival cadence, so
                    # piece 0 streams without per-dc stalls
                    pss = [
                        psum_s.tile([P, 2 * NB], f32, tag="ps", name="ps_gt")
                        for _ in range(2)
                    ]
                    for dc in range(DC):  # dc-pair groups match DMA sems
                        for hh in range(2):
                            for dq in range(2):
                                dpc = 2 * hh + dq
                                nc.tensor.matmul(
                                    pss[hh][:, dq * NB : (dq + 1) * NB],
                                    lhsT=Msb[:, dc, dpc * P : (dpc + 1) * P],
                                    rhs=ET[:, dc, pc * PW : (pc + 1) * PW],
                                    start=(dc == 0),
                                    stop=(dc == DC - 1),
                                )
                    for hh in range(2):
                        nc.vector.tensor_copy(
                            out=GT[
                                :, 2 * hh : 2 * hh + 2, pc * PW : (pc + 1) * PW
                            ],
                            in_=pss[hh].rearrange("p (k n) -> p k n", k=2),
                        )

                # ---- scores, software-pipelined with the GT pieces ----
                # stage q(mi, nb): one 512-wide matmul quartet; on the odd nb
                #   of each half, ACT exp(scale*s) -> fp16 + accum Z column
                # stage b(mi): 1/Z + normalize + row-block DMA
                # The first chunks run nb-quartet-granular between GT pieces
                # (a quartet only needs GT piece nb), hiding the XBAR DMA
                # cadence behind real PE work.
                ssbs: dict = {}
                apart: dict = {}
                psq: dict = {}

                def stage_q(mi, nb, qexp=False):
                    # qexp chunks exp after every quartet (4-col accum) so
                    # their PSUM drains immediately -> no pool deadlock when
                    # a chunk is left half-done across GT pieces
                    if mi not in apart:
                        ssb = exps.tile([P, N], f16, tag="ssb", name="ssb")
                        zp = small.tile(
                            [P, 4] if qexp else [P, 2],
                            f32,
                            tag="zp4" if qexp else "zp",
                            name="zp",
                        )
                        apart[mi] = (ssb, zp)
                    ssb, zp = apart[mi]
                    if qexp or nb % 2 == 0:
                        psq[mi] = psum_s.tile(
                            [P, 2 * NB], f32, tag="ps", name="ps_a"
                        )
                    ps = psq[mi]
                    half = nb % 2
                    for dc in range(DC):
                        nc.tensor.matmul(
                            ps[:, half * NB : (half + 1) * NB],
                            lhsT=ET[:, dc, mi * P : (mi + 1) * P],
                            rhs=GT[:, dc, nb * NB : (nb + 1) * NB],
                            start=(dc == 0),
                            stop=(dc == DC - 1),
                        )
                    if qexp:
                        nc.scalar.activation(
                            ssb[:, nb * NB : (nb + 1) * NB],
                            psq.pop(mi)[:, half * NB : (half + 1) * NB],
                            Exp,
                            scale=scale,
                            accum_out=zp[:, nb : nb + 1],
                        )
                    elif nb % 2 == 1:
                        h = nb // 2
                        nc.scalar.activation(
                            ssb[:, h * 2 * NB : (h + 1) * 2 * NB],
                            psq.pop(mi),
                            Exp,
                            scale=scale,
                            accum_out=zp[:, h : h + 1],
                        )

                def stage_a_half(mi, h):
                    stage_q(mi, 2 * h)
                    stage_q(mi, 2 * h + 1)

                def stage_a_fin(mi):
                    ssb, zp = apart.pop(mi)
                    zs = small.tile([P, 1], f32, tag="zs")
                    nc.vector.reduce_sum(zs, zp, axis=X)
                    rv = small.tile([P, 1], f32, tag="rv")
                    nc.vector.reciprocal(rv, zs)
                    ssbs[mi] = (ssb, rv)

                def stage_b(mi):
                    # normalize: single DVE tensor_scalar (f16 in/out packs to
                    # 4x mode; the fp32 per-partition scalar is allowed), then
                    # ship the strip as 16 4KB-contiguous rows of O^T.
                    ssb, rv = ssbs.pop(mi)
                    osb = outs.tile([P, N], f16, tag="osb")
                    nc.vector.tensor_scalar_mul(osb[:], ssb[:], rv)
                    nc.sync.dma_start(O_d[mi * P : (mi + 1) * P, :], osb)

                # chunks 0/1 run their first quartet between gt pieces 0
                # and 1, filling the ET piece-1 DMA latency bubble
                if _BUILD_PHASES[0] == 0:
                    gt_piece(0)
                    stage_q(0, 0, qexp=True)
                    stage_q(1, 0, qexp=True)
                    stage_q(2, 0, qexp=True)
                    stage_q(3, 0, qexp=True)
                    gt_piece(1)
                    gt_piece(2)
                    gt_piece(3)
                    for mi in range(4):
                        for nb in range(1, 4):
                            stage_q(mi, nb, qexp=True)
                        stage_a_fin(mi)
                        stage_b(mi)
                    for mi in range(4, MC - 1):
                        stage_a_half(mi, 0)
                        stage_a_half(mi, 1)
                        stage_a_fin(mi)
                        stage_b(mi)
                else:
                    for pc in range(NPC):
                        gt_piece(pc)

                def stage_tail():
                    # last chunk in shrinking segments (512x3 + 256x2): each
                    # segment's matmuls -> exp (own accum column); after z
                    # closes, normalize segments alternate DVE/ACT and DMAs
                    # split across the SP and ACT queues so the exposed chain
                    # after the final matmul is one 256-col segment deep
                    mi = MC - 1
                    rows = slice(mi * P, (mi + 1) * P)
                    segs = [(0, 768), (768, 768), (1536, 512)]
                    ssb = exps.tile([P, N], f16, tag="ssb", name="ssb_t")
                    zp = small.tile([P, 4], f32, tag="zp5", name="zp5")
                    for si, (c0, w) in enumerate(segs):
                        ps = psum_s.tile(
                            [P, 2 * NB], f32, tag="ps", name="ps_at"
                        )
                        # matmul moving dim caps at 512 (one PSUM bank); the
                        # exp still reads the whole segment in one pass
                        for b0 in range(0, w, NB):
                            bw = min(NB, w - b0)
                            for dc in range(DC):
                                nc.tensor.matmul(
                                    ps[:, b0 : b0 + bw],
                                    lhsT=ET[:, dc, rows],
                                    rhs=GT[:, dc, c0 + b0 : c0 + b0 + bw],
                                    start=(dc == 0),
                                    stop=(dc == DC - 1),
                                )
                        nc.scalar.activation(
                            ssb[:, c0 : c0 + w],
                            ps[:, 0:w],
                            Exp,
                            scale=scale,
                            accum_out=zp[:, si : si + 1],
                        )
                    zs = small.tile([P, 1], f32, tag="zs", name="zs_t")
                    nc.vector.reduce_sum(zs, zp[:, : len(segs)], axis=X)
                    rv = small.tile([P, 1], f32, tag="rv", name="rv_t")
                    nc.vector.reciprocal(rv, zs)
                    osb = outs.tile([P, N], f16, tag="osb", name="osb_t")
                    dma_eng = [nc.sync, nc.scalar, nc.sync]
                    for si, (c0, w) in enumerate(segs):
                        seg = slice(c0, c0 + w)
                        nc.vector.tensor_scalar_mul(
                            osb[:, seg], ssb[:, seg], rv
                        )
                        dma_eng[si].dma_start(O_d[rows, seg], osb[:, seg])

                if _BUILD_PHASES[0] == 0:
                    stage_tail()

            for _rep in range(repeat):
                emit_once()

    _split_multi_waits(nc)
    return nc


def _get_core(repeat=1):
    """Build (once) the Bass module and its I/O metadata."""
    if ("core", repeat) in _CACHE:
        return _CACHE[("core", repeat)]

    import jax

    import concourse.mybir as mybir
    from concourse import bass2jax

    nc = _build_nc(repeat)
    bass2jax.install_neuronx_cc_hook()

    partition_name = (
        nc.partition_id_tensor.name if nc.partition_id_tensor else None
    )

    in_names = []
    out_names = []
    out_avals = []
    for alloc in nc.m.functions[0].allocations:
        if not isinstance(alloc, mybir.MemoryLocationSet):
            continue
        name = alloc.memorylocations[0].name
        if alloc.kind == "ExternalInput":
            if name != partition_name:
                in_names.append(name)
        elif alloc.kind == "ExternalOutput":
            out_names.append(name)
            out_avals.append(
                jax.core.ShapedArray(
                    tuple(alloc.tensor_shape), mybir.dt.np(alloc.dtype)
                )
            )
    in_names_all = list(in_names) + list(out_names)
    if partition_name is not None:
        in_names_all.append(partition_name)

    _CACHE[("core", repeat)] = (
        nc, partition_name, in_names, out_names, out_avals, in_names_all
    )
    return _CACHE[("core", repeat)]


def _bind_exec(nc, partition_name, in_names_all, out_names, out_avals, operands):
    from concourse import bass2jax

    if partition_name is not None:
        operands = operands + [bass2jax.partition_id_tensor()]
    return tuple(
        bass2jax._bass_exec_p.bind(
            *operands,
            out_avals=tuple(out_avals),
            in_names=tuple(in_names_all),
            out_names=tuple(out_names),
            lowering_input_output_aliases=(),
            sim_require_finite=True,
            sim_require_nnan=True,
            nc=nc,
        )
    )


def _shard_jit(body, n_in, n_out):
    import jax
    import numpy as _np
    from jax.sharding import Mesh, PartitionSpec
    from jax.experimental.shard_map import shard_map

    devices = jax.devices()[:B]
    mesh = Mesh(_np.asarray(devices), ("core",))
    in_specs = (PartitionSpec("core"),) * n_in
    out_specs = (PartitionSpec("core"),) * n_out
    return jax.jit(
        shard_map(
            body, mesh=mesh, in_specs=in_specs, out_specs=out_specs, check_rep=False
        ),
        keep_unused=True,
    )


def _get_runner(repeat=1):
    """Jitted SPMD runner: fn(*args) -> concatenated outputs."""
    if ("runner", repeat) in _CACHE:
        return _CACHE[("runner", repeat)]

    import jax
    import numpy as _np

    nc, partition_name, in_names, out_names, out_avals, in_names_all = _get_core(repeat)
    n_params = len(in_names)
    n_outs = len(out_avals)

    def _body(*args):
        return _bind_exec(
            nc, partition_name, in_names_all, out_names, out_avals, list(args)
        )

    fn = _shard_jit(_body, n_params + n_outs, n_outs)

    def pack(in_maps):
        concat_in = [
            _np.concatenate([_np.asarray(m[name]) for m in in_maps], axis=0)
            for name in in_names
        ]
        concat_zero = [
            _np.zeros((B * a.shape[0], *a.shape[1:]), a.dtype) for a in out_avals
        ]
        return [jax.device_put(a) for a in concat_in + concat_zero]

    _CACHE[("runner", repeat)] = (fn, pack, out_names, out_avals)
    return _CACHE[("runner", repeat)]


def kernel(E, W1, W2):
    import ml_dtypes

    E = np.ascontiguousarray(np.asarray(E), dtype=np.float32)
    W1 = np.asarray(W1, dtype=np.float32)
    W2 = np.asarray(W2, dtype=np.float32)
    # Fold the two projections: scores = E (W1 W2^T) E^T. Done in float64 on
    # host for accuracy; negligible cost (512^3 FLOPs).
    Mw = (W1.astype(np.float64) @ W2.astype(np.float64).T).astype(np.float32)
    # Device datapath is bf16; cast on host so XBAR transpose DMAs (2-byte
    # dtype only) can deliver E^T straight out of the load.
    Ebf = E.astype(ml_dtypes.bfloat16)
    Mbf = np.ascontiguousarray(Mw.T).astype(ml_dtypes.bfloat16)

    fn, pack, out_names, out_avals = _get_runner()
    in_maps = [{"E": Ebf[b], "M": Mbf} for b in range(B)]
    args = pack(in_maps)
    outs = fn(*args)
    o = np.asarray(outs[0])  # [8*N, N] fp16; each [N, N] block is out[b]^T
    return o.reshape(B, N, N).transpose(0, 2, 1).astype(np.float32, order="C")


if __name__ == "__main__":
    rng = np.random.default_rng(0)
    E = rng.standard_normal((B, N, D), dtype=np.float32)
    W1 = rng.standard_normal((D, D), dtype=np.float32) * (2.0 / (D + D)) ** 0.5
    W2 = rng.standard_normal((D, D), dtype=np.float32) * (2.0 / (D + D)) ** 0.5
    out = kernel(E=E, W1=W1, W2=W2)
    print(out.shape, out.dtype, out.sum())



# revision 58
# speedup vs baseline: 1.3075x; 1.0183x over previous
"""Trainium2 Bass kernel for nn_Attn_52432960749709.

Computes, for E:[B,N,D], W1/W2:[D,D]:
    q = E @ W1 ; k = E @ W2
    scores = (q @ k^T) / sqrt(D)          # per batch, [N, N]
    out = softmax(scores, axis=1)         # normalize over rows n, per column m

Strategy (data parallel over B across 8 NeuronCores, one batch element per
core; the small DxD weights are folded on the host into M = W1 @ W2^T and
replicated):

    scores = E M E^T / sqrt(D)
    Per core (one NeuronCore per batch element), all-bf16 datapath (host
    pre-casts E and ships M^T; tolerance is 2e-2, measured ~7e-3):
      head    14 PE warmup transposes (garbage data) burn the ~3us clock-ramp
              window so every real matmul runs at the full 2.4 GHz
      E^T, M  XBAR transpose DMAs deliver E^T and M straight out of HBM
              (14ns per 16x128 tile, ~12x cheaper than copy descriptors) —
              zero PE/ACT/Pool work. All loads stay on the one SP queue,
              which issues one DMA per ~0.65us, so slots are precious: M as
              two dc-pair 3D XBARs bracketing ET piece-0's per-dc pieces,
              ET piece-1 as dc-pairs, pieces 2-3 as one 3D DMA each.
              HW gotchas baked in here: (a) mixing copy- and transpose-
              flavor DMAs on one queue chains them on completion semaphores
              (+2.5us each); (b) ACT-queue-issued XBAR DMAs silently corrupt
              data on hardware (sim-only feature, it seems).
      G^T     G^T = M E^T as 4 512-column pieces, dc-outermost so the 4
              matmuls per dc match the per-dc DMA arrival cadence; PSUM ->
              bf16 SBUF copies on DVE. The first quartet of scores matmuls
              for chunks 0-3 (with per-quartet exp into a 4-column accum)
              interleaves between GT pieces to cover DMA latency.
      s^T     per m-chunk [128 m, 2048 n]: 16 bf16 matmuls (512-wide — the
              PSUM-bank limit); ACT exp(scale*s) -> fp16 strip per 1024-half
              with accum_out building Z per partition; DVE: 1/Z then one
              4x-packed tensor_scalar normalize
      out     the normalized strip IS a row block of out^T = softmax(s^T)
              along its free axis, so it DMAs straight to HBM as
              4KB-contiguous rows of O^T — no transpose-back
      tail    the last chunk runs in shrinking segments (768/768/512):
              exp per segment, and after Z closes, DVE normalizes while the
              SP and ACT queues split the three segment DMAs
    The device emits O^T per batch element; the host transposes the last two
    axes while upcasting fp16 -> fp32 during the unshard (pure layout work).
"""

import math

import numpy as np

B, N, D = 8, 2048, 512
P = 128
DC = D // P  # 4 contraction chunks
NB = 512  # matmul moving free dim
NBS = N // NB  # 4 n-blocks per row strip
MC = N // P  # 16 m-chunks per core

_CACHE: dict = {}

# debug: limit build to first K phases (0=all): 1=loads+ET, 2=+GT
_BUILD_PHASES = [0]


def _patch_tile_drain():
    """This walrus build rejects >1 extra sem wait on one TPB_CTRL
    instruction, so split the end-of-kernel drain's wait set across chained
    SP NOPs (same engine, so program order preserves barrier semantics)."""
    import concourse.tile as tile
    from concourse.vector_clock import ScopedClock

    if getattr(tile.TileContext, "_drain_split_patched", False):
        return

    max_waits = 1

    def _drain_and_barrier_split(self, tick_clock, wait_clock):
        nc = self.nc
        drain_inst = nc.sync.drain()
        wait_clock.add_sem_waits(
            drain_inst.ins, ScopedClock({None: tick_clock.global_clock})
        )
        si = drain_inst.ins.sync_info
        waits = list(si.on_wait or []) if si is not None else []
        if len(waits) > max_waits:
            si.on_wait = waits[:max_waits]
            rest = waits[max_waits:]
            while rest:
                nop = nc.sync.nop(nofuse=True, hint="drain_wait_split")
                chunk, rest = rest[:max_waits], rest[max_waits:]
                nsi = nop.ins.sync_info
                if nsi is None:
                    import bass_rust

                    nop.ins.sync_info = bass_rust.SyncInfo(
                        on_wait=chunk, on_update=[]
                    )
                else:
                    nsi.on_wait = chunk

        nc.all_engine_barrier()
        assert self.sems is not None
        popped = nc._tile_sem_poison_stack.pop()
        assert popped is self._sem_poison
        nc.clear_and_free_semaphores(list(self.sems.allocated().values()))
        nc.all_engine_barrier()

    tile.TileContext._drain_and_barrier = _drain_and_barrier_split
    tile.TileContext._drain_split_patched = True


def _split_multi_waits(nc):
    """This walrus build supports only one sem-wait command per instruction.
    Hoist extra waits onto same-engine NOPs inserted just before the
    instruction (engines execute in order, so semantics are preserved)."""
    import bass_rust
    import concourse.mybir as mybir

    ctr = 0
    for fn in nc.m.functions:
        for blk in fn.blocks:
            insts = blk.instructions
            out = []
            changed = False
            for inst in insts:
                si = inst.sync_info
                waits = list(si.on_wait) if (si is not None and si.on_wait) else []
                if len(waits) > 1:
                    changed = True
                    for w in waits[:-1]:
                        ctr += 1
                        nop = mybir.InstNoOp(name=f"I-waitsplit-{ctr}")
                        nop.engine = inst.engine
                        nop.sync_info = bass_rust.SyncInfo(
                            on_wait=[w], on_update=[]
                        )
                        nc.register_instruction(nop)
                        out.append(nop)
                    si.on_wait = waits[-1:]
                out.append(inst)
            if changed:
                blk.instructions = out


def _build_nc(repeat=1):
    import concourse.bass as bass
    import concourse.mybir as mybir
    import concourse.tile as tile
    from concourse.masks import make_identity

    _patch_tile_drain()

    dt = mybir.dt
    f32, f16, bf16 = dt.float32, dt.float16, dt.bfloat16
    Exp = mybir.ActivationFunctionType.Exp
    X = mybir.AxisListType.X

    scale = 1.0 / math.sqrt(float(D))

    nc = bass.Bass()
    # Host ships E and M pre-cast to bf16 (XBAR transpose DMA needs 2-byte
    # dtype; matmuls run bf16 anyway). Halves the load traffic too.
    E_d = nc.dram_tensor("E", [N, D], bf16, kind="ExternalInput")
    M_d = nc.dram_tensor("M", [D, D], bf16, kind="ExternalInput")
    # Holds out^T for this batch element: O[m, n] = softmax(s)[n, m].
    O_d = nc.dram_tensor("O", [N, N], f16, kind="ExternalOutput")

    with tile.TileContext(nc) as tc:
        with (
            tc.tile_pool(name="persist", bufs=1) as persist,
            tc.tile_pool(name="exps", bufs=8) as exps,
            tc.tile_pool(name="outs", bufs=3) as outs,
            tc.tile_pool(name="small", bufs=8) as small,
            tc.tile_pool(name="psum_s", bufs=4, space="PSUM") as psum_s,
        ):
            # warmup source: content irrelevant (transposes are throwaway);
            # a single Pool memset is the cheapest legal producer
            ident32 = persist.tile([P, P], f32, tag="id32")
            nc.gpsimd.memset(ident32, 0.0)

            # PE warmup: keep the PE busy while the first E tiles stream in,
            # so the HAM clock gate is released before real work arrives.
            # Borrows a scores-pool PSUM tile (contents are garbage; the next
            # user overwrites via a start=True matmul).
            warm = psum_s.tile([P, 2 * NB], f32, tag="ps", name="warm")
            for k in range(14):
                nc.tensor.transpose(
                    warm[:, (k % 8) * P : (k % 8 + 1) * P], ident32, ident32
                )

            # bf16 storage: single-pass PE matmuls; precision margin is ample
            # (tolerance 2e-2, measured ~2e-3).
            ET = persist.tile([P, DC, N], bf16, tag="ET")  # E^T  [d, n]
            GT = persist.tile([P, DC, N], bf16, tag="GT")  # G^T  [d', n]
            Msb = persist.tile([P, DC, D], bf16, tag="M")  # M    [d, d']

            def emit_once():
                # ---- E^T via XBAR transpose DMAs (14ns per 16x128 tile);
                # M loaded bf16 directly. GT = M E^T streams right behind the
                # DMAs: piece pc covers 512 n-columns; within a piece the
                # matmuls run dc-innermost to match DMA arrival order, and the
                # dc-chunk of M is queued just ahead of ET(piece0, dc).
                PW = NB  # 512-column ET/GT staging piece
                NPC = N // PW  # 4 pieces

                # All loads are XBAR transposes (M_d holds M^T and is
                # flipped back in-flight). Two rules shape this schedule:
                # (1) one DMA flavor per queue end-to-end — mixing copy and
                # transpose DMAs on a queue serializes them on completion
                # semaphores (+2.5us per transition); (2) each queue issues a
                # DMA only every ~0.65us, so the loads alternate between the
                # SP and ACT queues, in GT's consumption order, to keep the
                # DMA device streaming back-to-back.
                # per-dc front loads alternating between the SP and ACT
                # queues (each queue issues one DMA per ~0.65us), in GT's
                # consumption order: M chunk dc just ahead of ET(piece, dc)
                def ldt(dst, srcap):
                    nc.sync.dma_start_transpose(dst, srcap)

                # M as two dc-pair XBARs bracketing the first ET pieces:
                # frees two issue slots vs per-chunk M, pulling every later
                # ET piece ~1.3us earlier (the queue issues one DMA/~0.65us)
                ldt(Msb[:, 0:2, :], M_d[:, 0 : 2 * P])
                ldt(ET[:, 0, 0:PW], E_d[0:PW, 0:P])
                ldt(ET[:, 1, 0:PW], E_d[0:PW, P : 2 * P])
                ldt(Msb[:, 2:4, :], M_d[:, 2 * P : 4 * P])
                ldt(ET[:, 2, 0:PW], E_d[0:PW, 2 * P : 3 * P])
                ldt(ET[:, 3, 0:PW], E_d[0:PW, 3 * P : 4 * P])
                for c in range(2):
                    ldt(
                        ET[:, 2 * c : 2 * c + 2, PW : 2 * PW],
                        E_d[PW : 2 * PW, 2 * c * P : (2 * c + 2) * P],
                    )
                for pc in range(2, NPC):
                    ldt(
                        ET[:, :, pc * PW : (pc + 1) * PW],
                        E_d[pc * PW : (pc + 1) * PW, :],
                    )

                def gt_piece(pc):
                    # dc outermost across BOTH psum halves: 4 matmuls per dc
                    # (~0.85us) matches the per-dc XBAR arrival cadence, so
                    # piece 0 streams without per-dc stalls
                    pss = [
                        psum_s.tile([P, 2 * NB], f32, tag="ps", name="ps_gt")
                        for _ in range(2)
                    ]
                    for dc in range(DC):  # dc-pair groups match DMA sems
                        for hh in range(2):
                            for dq in range(2):
                                dpc = 2 * hh + dq
                                nc.tensor.matmul(
                                    pss[hh][:, dq * NB : (dq + 1) * NB],
                                    lhsT=Msb[:, dc, dpc * P : (dpc + 1) * P],
                                    rhs=ET[:, dc, pc * PW : (pc + 1) * PW],
                                    start=(dc == 0),
                                    stop=(dc == DC - 1),
                                )
                    for hh in range(2):
                        nc.vector.tensor_copy(
                            out=GT[
                                :, 2 * hh : 2 * hh + 2, pc * PW : (pc + 1) * PW
                            ],
                            in_=pss[hh].rearrange("p (k n) -> p k n", k=2),
                        )

                # ---- scores, software-pipelined with the GT pieces ----
                # stage q(mi, nb): one 512-wide matmul quartet; on the odd nb
                #   of each half, ACT exp(scale*s) -> fp16 + accum Z column
                # stage b(mi): 1/Z + normalize + row-block DMA
                # The first chunks run nb-quartet-granular between GT pieces
                # (a quartet only needs GT piece nb), hiding the XBAR DMA
                # cadence behind real PE work.
                ssbs: dict = {}
                apart: dict = {}
                psq: dict = {}

                def stage_q(mi, nb, qexp=False):
                    # qexp chunks exp after every quartet (4-col accum) so
                    # their PSUM drains immediately -> no pool deadlock when
                    # a chunk is left half-done across GT pieces
                    if mi not in apart:
                        ssb = exps.tile([P, N], f16, tag="ssb", name="ssb")
                        zp = small.tile(
                            [P, 4] if qexp else [P, 2],
                            f32,
                            tag="zp4" if qexp else "zp",
                            name="zp",
                        )
                        apart[mi] = (ssb, zp)
                    ssb, zp = apart[mi]
                    if qexp or nb % 2 == 0:
                        psq[mi] = psum_s.tile(
                            [P, 2 * NB], f32, tag="ps", name="ps_a"
                        )
                    ps = psq[mi]
                    half = nb % 2
                    for dc in range(DC):
                        nc.tensor.matmul(
                            ps[:, half * NB : (half + 1) * NB],
                            lhsT=ET[:, dc, mi * P : (mi + 1) * P],
                            rhs=GT[:, dc, nb * NB : (nb + 1) * NB],
                            start=(dc == 0),
                            stop=(dc == DC - 1),
                        )
                    if qexp:
                        nc.scalar.activation(
                            ssb[:, nb * NB : (nb + 1) * NB],
                            psq.pop(mi)[:, half * NB : (half + 1) * NB],
                            Exp,
                            scale=scale,
                            accum_out=zp[:, nb : nb + 1],
                        )
                    elif nb % 2 == 1:
                        h = nb // 2
                        nc.scalar.activation(
                            ssb[:, h * 2 * NB : (h + 1) * 2 * NB],
                            psq.pop(mi),
                            Exp,
                            scale=scale,
                            accum_out=zp[:, h : h + 1],
                        )

                def stage_a_half(mi, h):
                    stage_q(mi, 2 * h)
                    stage_q(mi, 2 * h + 1)

                def stage_a_fin(mi):
                    ssb, zp = apart.pop(mi)
                    zs = small.tile([P, 1], f32, tag="zs")
                    nc.vector.reduce_sum(zs, zp, axis=X)
                    rv = small.tile([P, 1], f32, tag="rv")
                    nc.vector.reciprocal(rv, zs)
                    ssbs[mi] = (ssb, rv)

                def stage_b(mi):
                    # normalize: single DVE tensor_scalar (f16 in/out packs to
                    # 4x mode; the fp32 per-partition scalar is allowed), then
                    # ship the strip as 16 4KB-contiguous rows of O^T.
                    ssb, rv = ssbs.pop(mi)
                    osb = outs.tile([P, N], f16, tag="osb")
                    nc.vector.tensor_scalar_mul(osb[:], ssb[:], rv)
                    nc.sync.dma_start(O_d[mi * P : (mi + 1) * P, :], osb)

                # chunks 0/1 run their first quartet between gt pieces 0
                # and 1, filling the ET piece-1 DMA latency bubble
                if _BUILD_PHASES[0] == 0:
                    gt_piece(0)
                    gt_piece(1)
                    stage_q(0, 0, qexp=True)
                    stage_q(1, 0, qexp=True)
                    stage_q(2, 0, qexp=True)
                    stage_q(3, 0, qexp=True)
                    gt_piece(2)
                    gt_piece(3)
                    for mi in range(4):
                        for nb in range(1, 4):
                            stage_q(mi, nb, qexp=True)
                        stage_a_fin(mi)
                        stage_b(mi)
                    for mi in range(4, MC - 1):
                        stage_a_half(mi, 0)
                        stage_a_half(mi, 1)
                        stage_a_fin(mi)
                        stage_b(mi)
                else:
                    for pc in range(NPC):
                        gt_piece(pc)

                def stage_tail():
                    # last chunk in shrinking segments (512x3 + 256x2): each
                    # segment's matmuls -> exp (own accum column); after z
                    # closes, normalize segments alternate DVE/ACT and DMAs
                    # split across the SP and ACT queues so the exposed chain
                    # after the final matmul is one 256-col segment deep
                    mi = MC - 1
                    rows = slice(mi * P, (mi + 1) * P)
                    segs = [(0, 768), (768, 768), (1536, 512)]
                    ssb = exps.tile([P, N], f16, tag="ssb", name="ssb_t")
                    zp = small.tile([P, 4], f32, tag="zp5", name="zp5")
                    for si, (c0, w) in enumerate(segs):
                        ps = psum_s.tile(
                            [P, 2 * NB], f32, tag="ps", name="ps_at"
                        )
                        # matmul moving dim caps at 512 (one PSUM bank); the
                        # exp still reads the whole segment in one pass
                        for b0 in range(0, w, NB):
                            bw = min(NB, w - b0)
                            for dc in range(DC):
                                nc.tensor.matmul(
                                    ps[:, b0 : b0 + bw],
                                    lhsT=ET[:, dc, rows],
                                    rhs=GT[:, dc, c0 + b0 : c0 + b0 + bw],
                                    start=(dc == 0),
                                    stop=(dc == DC - 1),
                                )
                        nc.scalar.activation(
                            ssb[:, c0 : c0 + w],
                            ps[:, 0:w],
                            Exp,
                            scale=scale,
                            accum_out=zp[:, si : si + 1],
                        )
                    zs = small.tile([P, 1], f32, tag="zs", name="zs_t")
                    nc.vector.reduce_sum(zs, zp[:, : len(segs)], axis=X)
                    rv = small.tile([P, 1], f32, tag="rv", name="rv_t")
                    nc.vector.reciprocal(rv, zs)
                    osb = outs.tile([P, N], f16, tag="osb", name="osb_t")
                    dma_eng = [nc.sync, nc.scalar, nc.sync]
                    for si, (c0, w) in enumerate(segs):
                        seg = slice(c0, c0 + w)
                        nc.vector.tensor_scalar_mul(
                            osb[:, seg], ssb[:, seg], rv
                        )
                        dma_eng[si].dma_start(O_d[rows, seg], osb[:, seg])

                if _BUILD_PHASES[0] == 0:
                    stage_tail()

            for _rep in range(repeat):
                emit_once()

    _split_multi_waits(nc)
    return nc


def _get_core(repeat=1):
    """Build (once) the Bass module and its I/O metadata."""
    if ("core", repeat) in _CACHE:
        return _CACHE[("core", repeat)]

    import jax

    import concourse.mybir as mybir
    from concourse import bass2jax

    nc = _build_nc(repeat)
    bass2jax.install_neuronx_cc_hook()

    partition_name = (
        nc.partition_id_tensor.name if nc.partition_id_tensor else None
    )

    in_names = []
    out_names = []
    out_avals = []
    for alloc in nc.m.functions[0].allocations:
        if not isinstance(alloc, mybir.MemoryLocationSet):
            continue
        name = alloc.memorylocations[0].name
        if alloc.kind == "ExternalInput":
            if name != partition_name:
                in_names.append(name)
        elif alloc.kind == "ExternalOutput":
            out_names.append(name)
            out_avals.append(
                jax.core.ShapedArray(
                    tuple(alloc.tensor_shape), mybir.dt.np(alloc.dtype)
                )
            )
    in_names_all = list(in_names) + list(out_names)
    if partition_name is not None:
        in_names_all.append(partition_name)

    _CACHE[("core", repeat)] = (
        nc, partition_name, in_names, out_names, out_avals, in_names_all
    )
    return _CACHE[("core", repeat)]


def _bind_exec(nc, partition_name, in_names_all, out_names, out_avals, operands):
    from concourse import bass2jax

    if partition_name is not None:
        operands = operands + [bass2jax.partition_id_tensor()]
    return tuple(
        bass2jax._bass_exec_p.bind(
            *operands,
            out_avals=tuple(out_avals),
            in_names=tuple(in_names_all),
            out_names=tuple(out_names),
            lowering_input_output_aliases=(),
            sim_require_finite=True,
            sim_require_nnan=True,
            nc=nc,
        )
    )


def _shard_jit(body, n_in, n_out):
    import jax
    import numpy as _np
    from jax.sharding import Mesh, PartitionSpec
    from jax.experimental.shard_map import shard_map

    devices = jax.devices()[:B]
    mesh = Mesh(_np.asarray(devices), ("core",))
    in_specs = (PartitionSpec("core"),) * n_in
    out_specs = (PartitionSpec("core"),) * n_out
    return jax.jit(
        shard_map(
            body, mesh=mesh, in_specs=in_specs, out_specs=out_specs, check_rep=False
        ),
        keep_unused=True,
    )


def _get_runner(repeat=1):
    """Jitted SPMD runner: fn(*args) -> concatenated outputs."""
    if ("runner", repeat) in _CACHE:
        return _CACHE[("runner", repeat)]

    import jax
    import numpy as _np

    nc, partition_name, in_names, out_names, out_avals, in_names_all = _get_core(repeat)
    n_params = len(in_names)
    n_outs = len(out_avals)

    def _body(*args):
        return _bind_exec(
            nc, partition_name, in_names_all, out_names, out_avals, list(args)
        )

    fn = _shard_jit(_body, n_params + n_outs, n_outs)

    def pack(in_maps):
        concat_in = [
            _np.concatenate([_np.asarray(m[name]) for m in in_maps], axis=0)
            for name in in_names
        ]
        concat_zero = [
            _np.zeros((B * a.shape[0], *a.shape[1:]), a.dtype) for a in out_avals
        ]
        return [jax.device_put(a) for a in concat_in + concat_zero]

    _CACHE[("runner", repeat)] = (fn, pack, out_names, out_avals)
    return _CACHE[("runner", repeat)]


def kernel(E, W1, W2):
    import ml_dtypes

    E = np.ascontiguousarray(np.asarray(E), dtype=np.float32)
    W1 = np.asarray(W1, dtype=np.float32)
    W2 = np.asarray(W2, dtype=np.float32)
    # Fold the two projections: scores = E (W1 W2^T) E^T. Done in float64 on
    # host for accuracy; negligible cost (512^3 FLOPs).
    Mw = (W1.astype(np.float64) @ W2.astype(np.float64).T).astype(np.float32)
    # Device datapath is bf16; cast on host so XBAR transpose DMAs (2-byte
    # dtype only) can deliver E^T straight out of the load.
    Ebf = E.astype(ml_dtypes.bfloat16)
    Mbf = np.ascontiguousarray(Mw.T).astype(ml_dtypes.bfloat16)

    fn, pack, out_names, out_avals = _get_runner()
    in_maps = [{"E": Ebf[b], "M": Mbf} for b in range(B)]
    args = pack(in_maps)
    outs = fn(*args)
    o = np.asarray(outs[0])  # [8*N, N] fp16; each [N, N] block is out[b]^T
    return o.reshape(B, N, N).transpose(0, 2, 1).astype(np.float32, order="C")


if __name__ == "__main__":
    rng = np.random.default_rng(0)
    E = rng.standard_normal((B, N, D), dtype=np.float32)
    W1 = rng.standard_normal((D, D), dtype=np.float32) * (2.0 / (D + D)) ** 0.5
    W2 = rng.standard_normal((D, D), dtype=np.float32) * (2.0 / (D + D)) ** 0.5
    out = kernel(E=E, W1=W1, W2=W2)
    print(out.shape, out.dtype, out.sum())

